# revision 1
# baseline (speedup 1.0000x reference)
"""GAT (3-layer, PyG GATConv semantics) on 8 Trainium2 NeuronCores.

Strategy (dst-node sharding, per the standard graph-parallel recipe):
- Nodes padded to 50176 = 8 * 6272; core c owns dst rows [c*6272, (c+1)*6272).
- Per layer:
  dense (sharded): table_shard = h_shard @ [W | W@a_src | W@a_dst] in bf16,
      via HW DMA-transpose loads of h^T tiles; rows written at a
      256B-multiple pitch so dma_gather can index them.
  AllGather the node table so every core can fetch arbitrary src rows.
  edge phase: edges bucketed by dst into 128-slot blocks (padded to uniform
      sub-tile counts so the SPMD program is identical on all cores).
      Per block: batched dma_gather of src rows (int16 indices, split into
      lo/hi halves because indices are signed 16-bit), one-hot selection
      matrices via is_equal(iota, dst_local), segment softmax WITHOUT the
      max-subtraction (attention logits here are O(0.3), so exp is safe and
      the result is mathematically identical), ex-weighted features + ex
      accumulated into PSUM with one matmul per 128-edge sub-tile, then the
      per-node normalize / bias / BN / activation fused on DVE+ACT.
- d[dst] per edge is produced on-chip: transpose the one-hot with the PE and
  multiply against the block's d rows (avoids a second descriptor-bound
  gather; descriptors are the bottleneck resource on this hardware).
"""
import os
import sys
import types

sys.path.insert(0, "/opt/trn_rl_repo")

import numpy as np
import ml_dtypes


def _install_ntff_shim():
    """Provide antenv.axon_hooks so run_bass_kernel_spmd(trace=True) works."""
    try:
        import antenv

        if "antenv.axon_hooks" in sys.modules:
            return
        mod = types.ModuleType("antenv.axon_hooks")
        mod._hook = None
        mod.set_axon_ntff_profile_hook = lambda h: setattr(mod, "_hook", h)
        mod.get_axon_ntff_profile_hook = lambda: mod._hook
        sys.modules["antenv.axon_hooks"] = mod
        antenv.axon_hooks = mod
        from trn_agent_boot.trn_boot import _ntff_profile_via_ctypes

        hook = _ntff_profile_via_ctypes("/opt/axon/libaxon_pjrt.so")
        if hook is not None:
            mod.set_axon_ntff_profile_hook(hook)
    except Exception:
        pass


_install_ntff_shim()

import concourse.bass as bass
import concourse.bacc as bacc
import concourse.mybir as mybir
import concourse.tile as tile
from concourse.bass_utils import run_bass_kernel_spmd
from concourse.masks import make_identity

bfnp = ml_dtypes.bfloat16
f32 = mybir.dt.float32
bf16 = mybir.dt.bfloat16
i16 = mybir.dt.int16
AF = mybir.ActivationFunctionType
OP = mybir.AluOpType

N, E = 50000, 800000
DIN, HID, HEADS, DOUT = 128, 32, 8, 16
NEG = 0.2
EPS = 1e-5

NCORES = 8
SHARD = 6272
NPAD = NCORES * SHARD  # 50176
NBLK = SHARD // 128  # 49
NLO, NHI = 10, 9  # sub-tiles per block: lo reads table[0:32768], hi reads table[17408:]
NSUB = NLO + NHI
LOCUT = 32768
HI_BASE = NPAD - LOCUT  # 17408

# table row pitches (bf16 cols; byte pitch must be a multiple of 256)
DROW12, USED12 = 384, 272  # [xw(256) | s(8) | d(8) | pad]
DROW3, USED3 = 128, 18  # [xw(16) | s(1) | d(1) | pad]
PAD_DSTL = 200.0  # one-hot miss marker for padding edges


def _build_Wp(W, a_s, a_d):
    H, F = a_s.shape
    Ws = np.stack([W[:, h * F : (h + 1) * F] @ a_s[h] for h in range(H)], axis=1)
    Wd = np.stack([W[:, h * F : (h + 1) * F] @ a_d[h] for h in range(H)], axis=1)
    return np.concatenate([W, Ws, Wd], axis=1).astype(bfnp)


def _wrap_idx(flat):
    """int16 list -> [128, len/16] wrapped in 16 partitions, replicated x8."""
    n = len(flat)
    assert n % 16 == 0
    w = flat.reshape(n // 16, 16).T  # [16, n/16]
    return np.tile(w, (8, 1)).astype(np.int16)


def _prep_edges(edge_src, edge_dst):
    """Bucket edges by (core, block); pack per-core gather indices and
    dst_local arrays with uniform [NBLK, ...] shapes."""
    src = np.concatenate([edge_src, np.arange(N, dtype=np.int32)])
    dst = np.concatenate([edge_dst, np.arange(N, dtype=np.int32)])
    core = dst // SHARD
    blk = (dst % SHARD) // 128
    out = []
    for c in range(NCORES):
        idx_lo = np.zeros((NBLK, 128, NLO * 8), np.int16)
        idx_hi = np.zeros((NBLK, 128, NHI * 8), np.int16)
        dstl = np.full((NBLK, 128, NSUB), PAD_DSTL, np.float32)
        m = core == c
        for b in range(NBLK):
            mb = m & (blk == b)
            s_all, d_all = src[mb], dst[mb]
            # balanced split: src<HI_BASE must go lo, src>=LOCUT must go hi,
            # middle goes to lo until lo is full
            order = np.argsort(s_all, kind="stable")
            s_all, d_all = s_all[order], d_all[order]
            ncut = min(int((s_all < LOCUT).sum()), NLO * 128)
            lo_s, lo_d = s_all[:ncut], d_all[:ncut]
            hi_s, hi_d = s_all[ncut:] - HI_BASE, d_all[ncut:]
            assert ncut == len(s_all) or s_all[ncut] >= HI_BASE
            for half, (ss, dd, cap, off) in enumerate(
                (
                    (lo_s, lo_d, NLO, 0),
                    (hi_s, hi_d, NHI, NLO),
                )
            ):
                n = len(ss)
                assert n <= cap * 128, (c, b, half, n)
                sp = np.zeros(cap * 128, np.int16)
                sp[:n] = ss.astype(np.int16)
                dl = np.full(cap * 128, PAD_DSTL, np.float32)
                dl[:n] = (dd - c * SHARD - b * 128).astype(np.float32)
                w = _wrap_idx(sp)
                if half == 0:
                    idx_lo[b] = w
                else:
                    idx_hi[b] = w
                # gathered row i lands at [p=i%128, j=i//128]
                dstl[b, :, off : off + cap] = dl.reshape(cap, 128).T
        out.append(
            {
                "idx_lo": idx_lo,
                "idx_hi": idx_hi,
                "dstl": dstl.astype(bfnp),
            }
        )
    return out


def _dense_phase(nc, tc, sb, ps, h_in, w_tiles, table_shard, d_own, used, h_cols):
    """table_shard[:, :used] = h_in @ W' ; d_own = last H cols. h_in is a
    DRAM [SHARD, h_cols] bf16 tensor (row-major)."""
    nk = h_cols // 128
    for t in range(NBLK):
        psd = ps.tile([128, used], f32, tag="psd", bufs=2)
        for k in range(nk):
            ht = sb.tile([128, 128], bf16, tag="ht", bufs=3)
            nc.sync.dma_start(
                out=ht[:],
                in_=h_in[t * 128 : (t + 1) * 128, k * 128 : (k + 1) * 128],
                transpose=True,
            )
            nc.tensor.matmul(
                out=psd[:],
                lhsT=ht[:],
                rhs=w_tiles[k][:],
                start=(k == 0),
                stop=(k == nk - 1),
            )
        row = sb.tile([128, used], bf16, tag="drow", bufs=3)
        nc.vector.tensor_copy(out=row[:], in_=psd[:])
        nc.sync.dma_start(
            out=table_shard[t * 128 : (t + 1) * 128, :used], in_=row[:]
        )
        nd = d_own.shape[1]
        nc.sync.dma_start(
            out=d_own[t * 128 : (t + 1) * 128, :], in_=row[:, used - nd : used]
        )


def _edge_phase(
    nc,
    tc,
    sb,
    ps,
    lyr,
    table_full,
    d_own,
    t_idx_lo,
    t_idx_hi,
    t_dstl,
    iota_t,
    ident_t,
    H,
    F,
    drow,
    bn_a_t,
    bn_c_t,
    h_out,
    final,
):
    HF = H * F
    rcols = HF + H  # matmul rhs cols: [gw | ex]
    for b in range(NBLK):
        qlo, qhi = (2 * b) % 4, (2 * b + 1) % 4
        ilo = sb.tile([128, NLO * 8], i16, tag="ilo", bufs=4)
        nc.sync.dma_start(out=ilo[:], in_=t_idx_lo[b])
        ihi = sb.tile([128, NHI * 8], i16, tag="ihi", bufs=4)
        nc.sync.dma_start(out=ihi[:], in_=t_idx_hi[b])
        dstl = sb.tile([128, NSUB], bf16, tag="dstl", bufs=3)
        nc.sync.dma_start(out=dstl[:], in_=t_dstl[b])
        dblk = sb.tile([128, H], bf16, tag="dblk", bufs=3)
        nc.sync.dma_start(out=dblk[:], in_=d_own[b * 128 : (b + 1) * 128, :])

        G = sb.tile([128, NSUB, drow], bf16, tag="G", bufs=3)
        nc.gpsimd.dma_gather(
            out_ap=G[:, 0:NLO, :],
            in_ap=table_full[0:LOCUT, :],
            idxs_ap=ilo[:16, :],
            num_idxs=NLO * 128,
            num_idxs_reg=NLO * 128,
            elem_size=drow,
            single_packet=False,
            queue_num=qlo,
        )
        nc.gpsimd.dma_gather(
            out_ap=G[:, NLO:NSUB, :],
            in_ap=table_full[HI_BASE:, :],
            idxs_ap=ihi[:16, :],
            num_idxs=NHI * 128,
            num_idxs_reg=NHI * 128,
            elem_size=drow,
            single_packet=False,
            queue_num=qhi,
        )

        # one-hot selection matrices, U[e, j, slot] = (dst_local[e, j] == slot)
        U = sb.tile([128, NSUB, 128], bf16, tag="U", bufs=2)
        nc.vector.tensor_tensor(
            out=U[:],
            in0=iota_t[:, None, :].to_broadcast([128, NSUB, 128]),
            in1=dstl[:, :, None].to_broadcast([128, NSUB, 128]),
            op=OP.is_equal,
        )
        # transposed one-hots (for d expansion), via PE transpose in groups of 4
        Ut = sb.tile([128, NSUB, 128], bf16, tag="Ut", bufs=2)
        for g in range(0, NSUB, 4):
            n = min(4, NSUB - g)
            pst = ps.tile([128, 512], bf16, tag="pst", bufs=2)
            for k in range(n):
                nc.tensor.transpose(
                    out=pst[:, k * 128 : (k + 1) * 128],
                    in_=U[:, g + k, :],
                    identity=ident_t[:],
                )
            nc.vector.tensor_copy(
                out=Ut[:, g : g + n, :],
                in_=pst[:, : n * 128].rearrange("p (j e) -> p j e", j=n),
            )
        # d per edge: d_pe[e, h] = d_blk[dst_local[e], h]
        psd = ps.tile([128, NSUB * H], f32, tag="psdpe", bufs=2)
        for j in range(NSUB):
            nc.tensor.matmul(
                out=psd[:, j * H : (j + 1) * H],
                lhsT=Ut[:, j, :],
                rhs=dblk[:],
                start=True,
                stop=True,
            )
        # ex = exp(leaky_relu(s_src + d_dst))
        alpha = sb.tile([128, NSUB * H], f32, tag="alpha", bufs=2)
        nc.vector.tensor_tensor(
            out=alpha[:].rearrange("p (j h) -> p j h", j=NSUB),
            in0=G[:, :, HF : HF + H],
            in1=psd[:].rearrange("p (j h) -> p j h", j=NSUB),
            op=OP.add,
        )
        # exp(leaky_relu(x)) == max(exp(x), exp(0.2 x)) -- keeps the ACT
        # engine on one function table (Exp) for the whole layer
        ex1 = sb.tile([128, NSUB * H], f32, tag="ex1", bufs=2)
        nc.scalar.activation(out=ex1[:], in_=alpha[:], func=AF.Exp)
        ex2 = sb.tile([128, NSUB * H], f32, tag="ex2", bufs=2)
        nc.scalar.activation(out=ex2[:], in_=alpha[:], func=AF.Exp, scale=NEG)
        exf = sb.tile([128, NSUB * H], f32, tag="exf", bufs=2)
        nc.vector.tensor_tensor(out=exf[:], in0=ex1[:], in1=ex2[:], op=OP.max)
        exb = sb.tile([128, NSUB * H], bf16, tag="exb", bufs=2)
        nc.vector.tensor_copy(out=exb[:], in_=exf[:])
        # gw = xw * ex (in place on G), ex into the s columns
        gview = G[:, :, 0:HF].rearrange("p j (h f) -> p j h f", h=H)
        nc.vector.tensor_tensor(
            out=gview,
            in0=gview,
            in1=exb[:].rearrange("p (j h) -> p j h", j=NSUB)[
                :, :, :, None
            ].to_broadcast([128, NSUB, H, F]),
            op=OP.mult,
        )
        nc.vector.tensor_copy(
            out=G[:, :, HF : HF + H],
            in_=exb[:].rearrange("p (j h) -> p j h", j=NSUB),
        )
        # accumulate [num | den] over the block's sub-tiles
        psa = ps.tile([128, rcols], f32, tag="psa", bufs=2)
        for j in range(NSUB):
            nc.tensor.matmul(
                out=psa[:],
                lhsT=U[:, j, :],
                rhs=G[:, j, 0:rcols],
                start=(j == 0),
                stop=(j == NSUB - 1),
            )
        # normalize + affine + activation
        den = sb.tile([128, H], f32, tag="den", bufs=2)
        nc.vector.tensor_scalar_add(out=den[:], in0=psa[:, HF : HF + H], scalar1=1e-30)
        rden = sb.tile([128, H], f32, tag="rden", bufs=2)
        nc.vector.reciprocal(out=rden[:], in_=den[:])
        o1 = sb.tile([128, HF], f32, tag="o1", bufs=2)
        nc.vector.tensor_tensor(
            out=o1[:].rearrange("p (h f) -> p h f", h=H),
            in0=psa[:, 0:HF].rearrange("p (h f) -> p h f", h=H),
            in1=rden[:].to_broadcast([128, H, F]),
            op=OP.mult,
        )
        o2 = sb.tile([128, HF], f32, tag="o2", bufs=2)
        nc.vector.tensor_tensor(out=o2[:], in0=o1[:], in1=bn_a_t[:], op=OP.mult)
        o3 = sb.tile([128, HF], f32, tag="o3", bufs=2)
        nc.vector.tensor_tensor(out=o3[:], in0=o2[:], in1=bn_c_t[:], op=OP.add)
        if final:
            outt = sb.tile([128, HF], f32, tag="outt", bufs=2)
            nc.scalar.activation(out=outt[:], in_=o3[:], func=AF.Sigmoid)
            nc.sync.dma_start(
                out=h_out[b * 128 : (b + 1) * 128, :], in_=outt[:]
            )
        else:
            # elu(x) = max(x, exp(min(x, 0)) - 1)
            e1 = sb.tile([128, HF], f32, tag="e1", bufs=2)
            nc.vector.tensor_scalar_min(out=e1[:], in0=o3[:], scalar1=0.0)
            e2 = sb.tile([128, HF], f32, tag="e2", bufs=2)
            nc.scalar.activation(out=e2[:], in_=e1[:], func=AF.Exp)
            e3 = sb.tile([128, HF], f32, tag="e3", bufs=2)
            nc.vector.tensor_scalar_add(out=e3[:], in0=e2[:], scalar1=-1.0)
            e4 = sb.tile([128, HF], f32, tag="e4", bufs=2)
            nc.vector.tensor_tensor(out=e4[:], in0=o3[:], in1=e3[:], op=OP.max)
            hb = sb.tile([128, HF], bf16, tag="hb", bufs=2)
            nc.vector.tensor_copy(out=hb[:], in_=e4[:])
            nc.sync.dma_start(out=h_out[b * 128 : (b + 1) * 128, :], in_=hb[:])


def _build_program():
    nc = bacc.Bacc(
        "TRN2",
        target_bir_lowering=False,
        debug=False,
        num_devices=NCORES,
        num_swdge_queues=4,
    )
    HD = HEADS * HID

    # --- inputs ---
    t_x = nc.dram_tensor("x_shard", [SHARD, DIN], bf16, kind="ExternalInput")
    t_w1 = nc.dram_tensor("W1p", [DIN, USED12], bf16, kind="ExternalInput")
    t_w2 = nc.dram_tensor("W2p", [HD, USED12], bf16, kind="ExternalInput")
    t_w3 = nc.dram_tensor("W3p", [HD, USED3], bf16, kind="ExternalInput")
    t_a1 = nc.dram_tensor("bn_a1", [128, HD], f32, kind="ExternalInput")
    t_c1 = nc.dram_tensor("bn_c1", [128, HD], f32, kind="ExternalInput")
    t_a2 = nc.dram_tensor("bn_a2", [128, HD], f32, kind="ExternalInput")
    t_c2 = nc.dram_tensor("bn_c2", [128, HD], f32, kind="ExternalInput")
    t_a3 = nc.dram_tensor("bn_a3", [128, DOUT], f32, kind="ExternalInput")
    t_c3 = nc.dram_tensor("bn_c3", [128, DOUT], f32, kind="ExternalInput")
    t_iota = nc.dram_tensor("iota_bf", [128, 128], bf16, kind="ExternalInput")
    t_ilo = nc.dram_tensor("idx_lo", [NBLK, 128, NLO * 8], i16, kind="ExternalInput")
    t_ihi = nc.dram_tensor("idx_hi", [NBLK, 128, NHI * 8], i16, kind="ExternalInput")
    t_dstl = nc.dram_tensor("dstl", [NBLK, 128, NSUB], bf16, kind="ExternalInput")
    t_out = nc.dram_tensor("out_shard", [SHARD, DOUT], f32, kind="ExternalOutput")

    with tile.TileContext(nc) as tc:
        with (
            tc.tile_pool(name="sb", bufs=2) as sb,
            tc.tile_pool(name="ps", bufs=2, space="PSUM") as ps,
            tc.tile_pool(name="dram", bufs=1, space="DRAM") as dr,
        ):
            # DRAM intermediates (pool tiles so Tile tracks dependencies)
            tb1_shard = dr.tile([SHARD, DROW12], bf16, name="tb1_shard")
            tb1_full = dr.tile([NPAD, DROW12], bf16, addr_space="Shared", name="tb1_full")
            tb2_shard = dr.tile([SHARD, DROW12], bf16, name="tb2_shard")
            tb2_full = dr.tile([NPAD, DROW12], bf16, addr_space="Shared", name="tb2_full")
            tb3_shard = dr.tile([SHARD, DROW3], bf16, name="tb3_shard")
            tb3_full = dr.tile([NPAD, DROW3], bf16, addr_space="Shared", name="tb3_full")
            d1_own = dr.tile([SHARD, HEADS], bf16, name="d1_own")
            d2_own = dr.tile([SHARD, HEADS], bf16, name="d2_own")
            d3_own = dr.tile([SHARD, 1], bf16, name="d3_own")
            h2_own = dr.tile([SHARD, HD], bf16, name="h2_own")
            h3_own = dr.tile([SHARD, HD], bf16, name="h3_own")

            # constants
            iota_t = sb.tile([128, 128], bf16, tag="iota", bufs=1)
            nc.sync.dma_start(out=iota_t[:], in_=t_iota[:])
            ident_t = sb.tile([128, 128], bf16, tag="ident", bufs=1)
            make_identity(nc, ident_t[:])
            w1t = [sb.tile([128, USED12], bf16, tag="w1", bufs=1, name="w1t0")]
            nc.sync.dma_start(out=w1t[0][:], in_=t_w1[:])
            w2t = [sb.tile([128, USED12], bf16, tag=f"w2_{k}", bufs=1, name=f"w2t{k}") for k in range(2)]
            for k in range(2):
                nc.sync.dma_start(out=w2t[k][:], in_=t_w2[k * 128 : (k + 1) * 128, :])
            w3t = [sb.tile([128, USED3], bf16, tag=f"w3_{k}", bufs=1, name=f"w3t{k}") for k in range(2)]
            for k in range(2):
                nc.sync.dma_start(out=w3t[k][:], in_=t_w3[k * 128 : (k + 1) * 128, :])
            bn = {}
            for nm, t, w in (
                ("a1", t_a1, HD), ("c1", t_c1, HD), ("a2", t_a2, HD),
                ("c2", t_c2, HD), ("a3", t_a3, DOUT), ("c3", t_c3, DOUT),
            ):
                bt = sb.tile([128, w], f32, tag=f"bn{nm}", bufs=1, name=f"bn{nm}")
                nc.sync.dma_start(out=bt[:], in_=t[:])
                bn[nm] = bt

            rg = [list(range(NCORES))]

            # ---- layer 1 ----
            _dense_phase(nc, tc, sb, ps, t_x, w1t, tb1_shard, d1_own, USED12, DIN)
            nc.gpsimd.collective_compute(
                "AllGather", OP.bypass, replica_groups=rg,
                ins=[tb1_shard[:]], outs=[tb1_full[:]],
            )
            _edge_phase(
                nc, tc, sb, ps, 1, tb1_full, d1_own, t_ilo, t_ihi, t_dstl,
                iota_t, ident_t, HEADS, HID, DROW12, bn["a1"], bn["c1"],
                h2_own, final=False,
            )
            # ---- layer 2 ----
            _dense_phase(nc, tc, sb, ps, h2_own, w2t, tb2_shard, d2_own, USED12, HD)
            nc.gpsimd.collective_compute(
                "AllGather", OP.bypass, replica_groups=rg,
                ins=[tb2_shard[:]], outs=[tb2_full[:]],
            )
            _edge_phase(
                nc, tc, sb, ps, 2, tb2_full, d2_own, t_ilo, t_ihi, t_dstl,
                iota_t, ident_t, HEADS, HID, DROW12, bn["a2"], bn["c2"],
                h3_own, final=False,
            )
            # ---- layer 3 ----
            _dense_phase(nc, tc, sb, ps, h3_own, w3t, tb3_shard, d3_own, USED3, HD)
            nc.gpsimd.collective_compute(
                "AllGather", OP.bypass, replica_groups=rg,
                ins=[tb3_shard[:]], outs=[tb3_full[:]],
            )
            _edge_phase(
                nc, tc, sb, ps, 3, tb3_full, d3_own, t_ilo, t_ihi, t_dstl,
                iota_t, ident_t, 1, DOUT, DROW3, bn["a3"], bn["c3"],
                t_out, final=True,
            )

    nc.compile()
    return nc


_CACHED = {}


def kernel(**inputs):
    x = np.asarray(inputs["x"], np.float32)
    edge_src = np.asarray(inputs["edge_src"], np.int32)
    edge_dst = np.asarray(inputs["edge_dst"], np.int32)

    xp = np.zeros((NPAD, DIN), np.float32)
    xp[:N] = x
    xb = xp.astype(bfnp)

    W1p = _build_Wp(
        np.asarray(inputs["W1"], np.float32),
        np.asarray(inputs["as1"], np.float32),
        np.asarray(inputs["ad1"], np.float32),
    )
    W2p = _build_Wp(
        np.asarray(inputs["W2"], np.float32),
        np.asarray(inputs["as2"], np.float32),
        np.asarray(inputs["ad2"], np.float32),
    )
    W3p = _build_Wp(
        np.asarray(inputs["W3"], np.float32),
        np.asarray(inputs["as3"], np.float32),
        np.asarray(inputs["ad3"], np.float32),
    )

    def aff(g, v, b, m, be):
        a = np.asarray(g, np.float32) / np.sqrt(np.asarray(v, np.float32) + EPS)
        c = (np.asarray(b, np.float32) - np.asarray(m, np.float32)) * a + np.asarray(
            be, np.float32
        )
        return a, c

    a1, c1 = aff(inputs["g1"], inputs["v1"], inputs["b1"], inputs["m1"], inputs["be1"])
    a2, c2 = aff(inputs["g2"], inputs["v2"], inputs["b2"], inputs["m2"], inputs["be2"])
    a3 = np.ones(DOUT, np.float32)
    c3 = np.asarray(inputs["b3"], np.float32)

    edata = _prep_edges(edge_src, edge_dst)
    iota = np.tile(np.arange(128, dtype=np.float32), (128, 1)).astype(bfnp)

    if "nc" not in _CACHED:
        _CACHED["nc"] = _build_program()
    nc = _CACHED["nc"]

    def bcast(v):
        return np.tile(np.asarray(v, np.float32), (128, 1))

    in_maps = []
    for c in range(NCORES):
        in_maps.append(
            {
                "x_shard": xb[c * SHARD : (c + 1) * SHARD],
                "W1p": W1p,
                "W2p": W2p,
                "W3p": W3p,
                "bn_a1": bcast(a1),
                "bn_c1": bcast(c1),
                "bn_a2": bcast(a2),
                "bn_c2": bcast(c2),
                "bn_a3": bcast(a3),
                "bn_c3": bcast(c3),
                "iota_bf": iota,
                "idx_lo": edata[c]["idx_lo"],
                "idx_hi": edata[c]["idx_hi"],
                "dstl": edata[c]["dstl"],
            }
        )

    trace = bool(os.environ.get("GAT_TRACE"))
    res = run_bass_kernel_spmd(
        nc, in_maps, core_ids=list(range(NCORES)), trace=trace
    )
    if trace and res.exec_time_ns:
        print(f"HW exec time: {res.exec_time_ns} ns")
    out = np.concatenate([res.results[c]["out_shard"] for c in range(NCORES)], axis=0)
    return np.ascontiguousarray(out[:N]).astype(np.float32)



# revision 9
# speedup vs baseline: 1.5818x; 1.5818x over previous
"""GAT (3-layer, PyG GATConv semantics) on 8 Trainium2 NeuronCores.

Strategy (dst-node sharding):
- Nodes padded to 50176 = 8 * 6272; core c owns dst rows [c*6272, (c+1)*6272).
- Per layer:
  dense (sharded): table_shard = h_shard @ [W*bn_a | W@a_src | W@a_dst] in bf16
      (bn scale folded into W's feature columns on the host), written both at
      a 256B-multiple pitch (for dma_gather) and contiguous (for local reads).
  AllGather the padded table so every core can fetch arbitrary src rows.
  edge phase: edges bucketed by dst into 128-slot blocks. Self-loops are NOT
      gathered: their contribution comes from the local contiguous table via
      an identity-lhsT matmul. Remaining edges are packed lo/hi (int16 index
      range split), padded with trailing -1 indices which the gather ucode
      SKIPS -- the true per-block descriptor count is passed at runtime via
      value_load, so padding costs nothing on the DMA engines.
      Per block: batched dma_gather of src rows, one-hot selection matrices
      via is_equal(iota, dst_local), segment softmax without max-subtraction
      (logits are O(0.3)), ex-weighted features + ex accumulated into PSUM
      with one matmul per 128-edge sub-tile, then normalize / bn / activation
      fused on DVE+ACT (elu via one scalar_tensor_tensor).
- d[dst] per edge is produced on-chip: transpose the one-hot with the PE and
  multiply against the block's d rows.
"""
import os
import sys
import types

sys.path.insert(0, "/opt/trn_rl_repo")

import numpy as np
import ml_dtypes


def _install_ntff_shim():
    """Provide antenv.axon_hooks so run_bass_kernel_spmd(trace=True) works."""
    try:
        import antenv

        if "antenv.axon_hooks" in sys.modules:
            return
        mod = types.ModuleType("antenv.axon_hooks")
        mod._hook = None
        mod.set_axon_ntff_profile_hook = lambda h: setattr(mod, "_hook", h)
        mod.get_axon_ntff_profile_hook = lambda: mod._hook
        sys.modules["antenv.axon_hooks"] = mod
        antenv.axon_hooks = mod
        from trn_agent_boot.trn_boot import _ntff_profile_via_ctypes

        hook = _ntff_profile_via_ctypes("/opt/axon/libaxon_pjrt.so")
        if hook is not None:
            mod.set_axon_ntff_profile_hook(hook)
    except Exception:
        pass


_install_ntff_shim()

import concourse.bass as bass
import concourse.bacc as bacc
import concourse.mybir as mybir
import concourse.tile as tile
from concourse.bass_utils import run_bass_kernel_spmd
from concourse.masks import make_identity

bfnp = ml_dtypes.bfloat16
f32 = mybir.dt.float32
bf16 = mybir.dt.bfloat16
i16 = mybir.dt.int16
i32 = mybir.dt.int32
AF = mybir.ActivationFunctionType
OP = mybir.AluOpType

N, E = 50000, 800000
DIN, HID, HEADS, DOUT = 128, 32, 8, 16
NEG = 0.2
EPS = 1e-5

NCORES = 8
SHARD = 6272
NPAD = NCORES * SHARD  # 50176
NBLK = SHARD // 128  # 49
LOCUT = 32768
HI_BASE = NPAD - LOCUT  # 17408

# table row pitches (bf16 cols; byte pitch must be a multiple of 256)
DROW12, USED12 = 384, 272  # [xw(256) | s(8) | d(8) | pad]
DROW3, USED3 = 128, 18  # [xw(16) | s(1) | d(1) | pad]
PAD_DSTL = 200.0  # one-hot miss marker for padding edges
GBUFS = 4


def _build_Wp(W, a_s, a_d, bn_a):
    H, F = a_s.shape
    Ws = np.stack([W[:, h * F : (h + 1) * F] @ a_s[h] for h in range(H)], axis=1)
    Wd = np.stack([W[:, h * F : (h + 1) * F] @ a_d[h] for h in range(H)], axis=1)
    return np.concatenate([W * bn_a[None, :], Ws, Wd], axis=1).astype(bfnp)


def _wrap_idx(flat):
    """int16 list -> [128, len/16] wrapped in 16 partitions, replicated x8."""
    n = len(flat)
    assert n % 16 == 0
    w = flat.reshape(n // 16, 16).T  # [16, n/16]
    return np.tile(w, (8, 1)).astype(np.int16)


def _prep_edges(edge_src, edge_dst):
    """Bucket non-self-loop edges by (core, block); returns per-core packed
    meta arrays (idx_lo | idx_hi | dstl) plus per-block valid counts, and the
    (NLO, NHI) sub-tile split chosen from the data."""
    src = edge_src.astype(np.int64)
    dst = edge_dst.astype(np.int64)
    core = dst // SHARD
    blk = (dst % SHARD) // 128

    # per-(core, block) stats to pick NLO/NHI
    nlo_need, nhi_need = 0, 0
    per_block = {}
    for c in range(NCORES):
        m = core == c
        for b in range(NBLK):
            mb = m & (blk == b)
            s_all, d_all = src[mb], dst[mb]
            order = np.argsort(s_all, kind="stable")
            per_block[(c, b)] = (s_all[order], d_all[order])

    # feasibility: lo-mandatory (< HI_BASE) must fit NLO*128;
    # overflow into hi after filling lo must fit NHI*128.
    def feasible(nlo, nhi):
        for (c, b), (s_all, _) in per_block.items():
            lo_mand = int((s_all < HI_BASE).sum())
            lo_elig = int((s_all < LOCUT).sum())
            if lo_mand > nlo * 128:
                return False
            n_lo = min(lo_elig, nlo * 128)
            if len(s_all) - n_lo > nhi * 128:
                return False
        return True

    nsub = None
    for tot in range(15, 22):
        ok = [
            (nlo, tot - nlo)
            for nlo in range(tot // 2, tot)
            if feasible(nlo, tot - nlo)
        ]
        if ok:
            nsub = tot
            nlo_need, nhi_need = ok[0]
            break
    assert nsub is not None, "no feasible (NLO, NHI) split"
    NLO, NHI = nlo_need, nhi_need
    NSUB = NLO + NHI
    ML = NLO * 8 + NHI * 8 + NSUB  # meta cols (int16)

    out = []
    for c in range(NCORES):
        meta = np.zeros((NBLK, 128, ML), np.int16)
        counts = np.zeros((NBLK, 2), np.int32)
        for b in range(NBLK):
            s_all, d_all = per_block[(c, b)]
            ncut = min(int((s_all < LOCUT).sum()), NLO * 128)
            lo_s, lo_d = s_all[:ncut], d_all[:ncut]
            hi_s, hi_d = s_all[ncut:] - HI_BASE, d_all[ncut:]
            assert ncut == len(s_all) or s_all[ncut] >= HI_BASE
            dstl = np.full((NSUB * 128,), PAD_DSTL, np.float32)
            for half, (ss, dd, cap, off) in enumerate(
                ((lo_s, lo_d, NLO, 0), (hi_s, hi_d, NHI, NLO))
            ):
                n = len(ss)
                assert n <= cap * 128, (c, b, half, n)
                sp = np.full(cap * 128, -1, np.int16)
                sp[:n] = ss.astype(np.int16)
                dl = dstl[off * 128 : (off + cap) * 128]
                dl[:n] = (dd - c * SHARD - b * 128).astype(np.float32)
                if n == 0:  # keep the gather ucode's count >= 1
                    sp[0], n = 0, 1
                counts[b, half] = n
                w = _wrap_idx(sp)
                if half == 0:
                    meta[b, :, 0 : NLO * 8] = w
                else:
                    meta[b, :, NLO * 8 : NLO * 8 + NHI * 8] = w
            # gathered row i lands at [p=i%128, j=i//128]
            dst_slot = dstl.reshape(NSUB, 128).T.astype(bfnp)  # [128, NSUB]
            meta[b, :, NLO * 8 + NHI * 8 :] = dst_slot.view(np.int16)
        out.append({"meta": meta, "counts": counts.reshape(-1)})
    return out, NLO, NHI


def _dense_phase(nc, tc, sb, ps, h_in, w_tiles, table_shard, own_tab, used, h_cols):
    """table_shard[:, :used] = own_tab = h_in @ W'. h_in is a DRAM
    [SHARD, h_cols] bf16 tensor (row-major)."""
    nk = h_cols // 128
    for t in range(NBLK):
        psd = ps.tile([128, used], f32, tag="psd", bufs=2)
        for k in range(nk):
            ht = sb.tile([128, 128], bf16, tag="ht", bufs=3)
            nc.sync.dma_start(
                out=ht[:],
                in_=h_in[t * 128 : (t + 1) * 128, k * 128 : (k + 1) * 128],
                transpose=True,
            )
            nc.tensor.matmul(
                out=psd[:],
                lhsT=ht[:],
                rhs=w_tiles[k][:],
                start=(k == 0),
                stop=(k == nk - 1),
            )
        row = sb.tile([128, used], bf16, tag="drow", bufs=3)
        nc.scalar.activation(out=row[:], in_=psd[:], func=AF.Copy)
        nc.sync.dma_start(
            out=table_shard[t * 128 : (t + 1) * 128, :used], in_=row[:]
        )
        nc.sync.dma_start(out=own_tab[t * 128 : (t + 1) * 128, :], in_=row[:])


def _edge_phase(
    nc,
    tc,
    sb,
    ps,
    table_full,
    own_tab,
    t_meta,
    t_counts,
    counts_t,
    iota_t,
    ident_t,
    H,
    F,
    drow,
    used,
    bn_c_t,
    h_out,
    final,
    NLO,
    NHI,
):
    NSUB = NLO + NHI
    HF = H * F
    rcols = HF + H  # matmul rhs cols: [gw | ex]
    ML = NLO * 8 + NHI * 8 + NSUB

    # memset the gather buffers once per layer: slots skipped by the runtime
    # descriptor count keep stale data, which must be finite (never NaN).
    for i in range(GBUFS):
        g0 = sb.tile([128, NSUB, drow], bf16, tag="G", bufs=GBUFS)
        nc.vector.memset(g0[:], 0.0)

    r_lo = nc.gpsimd.alloc_register(f"cnt_lo_{nc.next_id()}")
    r_hi = nc.gpsimd.alloc_register(f"cnt_hi_{nc.next_id()}")
    for b in range(NBLK):
        qlo, qhi = (2 * b) % 4, (2 * b + 1) % 4
        meta = sb.tile([128, ML], i16, tag="meta", bufs=4)
        nc.sync.dma_start(out=meta[:], in_=t_meta[b])
        own = sb.tile([128, used], bf16, tag="own", bufs=3)
        nc.sync.dma_start(out=own[:], in_=own_tab[b * 128 : (b + 1) * 128, :])
        dstl = meta[:, NLO * 8 + NHI * 8 :].bitcast(bf16)

        nc.gpsimd.reg_load(r_lo, counts_t[0:1, 2 * b : 2 * b + 1])
        nc.gpsimd.reg_load(r_hi, counts_t[0:1, 2 * b + 1 : 2 * b + 2])
        cnt_lo, cnt_hi = r_lo, r_hi

        G = sb.tile([128, NSUB, drow], bf16, tag="G", bufs=GBUFS)
        nc.gpsimd.dma_gather(
            out_ap=G[:, 0:NLO, :],
            in_ap=table_full[0:LOCUT, :],
            idxs_ap=meta[:16, 0 : NLO * 8],
            num_idxs=NLO * 128,
            num_idxs_reg=cnt_lo,
            elem_size=drow,
            single_packet=False,
            queue_num=qlo,
        )
        nc.gpsimd.dma_gather(
            out_ap=G[:, NLO:NSUB, :],
            in_ap=table_full[HI_BASE:, :],
            idxs_ap=meta[:16, NLO * 8 : NLO * 8 + NHI * 8],
            num_idxs=NHI * 128,
            num_idxs_reg=cnt_hi,
            elem_size=drow,
            single_packet=False,
            queue_num=qhi,
        )

        # one-hot selection matrices, U[e, j, slot] = (dst_local[e, j] == slot)
        U = sb.tile([128, NSUB, 128], bf16, tag="U", bufs=2)
        nc.vector.tensor_tensor(
            out=U[:],
            in0=iota_t[:, None, :].to_broadcast([128, NSUB, 128]),
            in1=dstl[:, :, None].to_broadcast([128, NSUB, 128]),
            op=OP.is_equal,
        )
        # transposed one-hots (for d expansion), via PE transpose in groups of 4
        Ut = sb.tile([128, NSUB, 128], bf16, tag="Ut", bufs=2)
        for g in range(0, NSUB, 4):
            n = min(4, NSUB - g)
            pst = ps.tile([128, 512], bf16, tag="pst", bufs=2)
            for k in range(n):
                nc.tensor.transpose(
                    out=pst[:, k * 128 : (k + 1) * 128],
                    in_=U[:, g + k, :],
                    identity=ident_t[:],
                )
            nc.vector.tensor_copy(
                out=Ut[:, g : g + n, :],
                in_=pst[:, : n * 128].rearrange("p (j e) -> p j e", j=n),
            )
        # d per edge: d_pe[e, h] = d_blk[dst_local[e], h]
        dblk = own[:, HF + H : HF + 2 * H]
        psd = ps.tile([128, NSUB * H], f32, tag="psdpe", bufs=2)
        for j in range(NSUB):
            nc.tensor.matmul(
                out=psd[:, j * H : (j + 1) * H],
                lhsT=Ut[:, j, :],
                rhs=dblk,
                start=True,
                stop=True,
            )
        # ex = exp(leaky_relu(s_src + d_dst)) == max(exp(x), exp(0.2 x))
        alpha = sb.tile([128, NSUB * H], f32, tag="alpha", bufs=2)
        nc.vector.tensor_tensor(
            out=alpha[:].rearrange("p (j h) -> p j h", j=NSUB),
            in0=G[:, :, HF : HF + H],
            in1=psd[:].rearrange("p (j h) -> p j h", j=NSUB),
            op=OP.add,
        )
        ex1 = sb.tile([128, NSUB * H], bf16, tag="ex1", bufs=2)
        nc.scalar.activation(out=ex1[:], in_=alpha[:], func=AF.Exp)
        ex2 = sb.tile([128, NSUB * H], bf16, tag="ex2", bufs=2)
        nc.scalar.activation(out=ex2[:], in_=alpha[:], func=AF.Exp, scale=NEG)
        exb = sb.tile([128, NSUB * H], bf16, tag="exb", bufs=2)
        nc.vector.tensor_tensor(out=exb[:], in0=ex1[:], in1=ex2[:], op=OP.max)
        # rhs tile for the psa accumulation: [xw*ex | ex]
        gw = sb.tile([128, NSUB, rcols], bf16, tag="gw", bufs=2)
        nc.vector.tensor_tensor(
            out=gw[:, :, 0:HF].rearrange("p j (h f) -> p j h f", h=H),
            in0=G[:, :, 0:HF].rearrange("p j (h f) -> p j h f", h=H),
            in1=exb[:].rearrange("p (j h) -> p j h", j=NSUB)[
                :, :, :, None
            ].to_broadcast([128, NSUB, H, F]),
            op=OP.mult,
        )
        nc.vector.tensor_copy(
            out=gw[:, :, HF:rcols],
            in_=exb[:].rearrange("p (j h) -> p j h", j=NSUB),
        )
        # self-loop contribution from the local contiguous table
        alph_s = sb.tile([128, H], f32, tag="alphs", bufs=2)
        nc.vector.tensor_tensor(
            out=alph_s[:],
            in0=own[:, HF : HF + H],
            in1=own[:, HF + H : HF + 2 * H],
            op=OP.add,
        )
        ex1s = sb.tile([128, H], bf16, tag="ex1s", bufs=2)
        nc.scalar.activation(out=ex1s[:], in_=alph_s[:], func=AF.Exp)
        ex2s = sb.tile([128, H], bf16, tag="ex2s", bufs=2)
        nc.scalar.activation(out=ex2s[:], in_=alph_s[:], func=AF.Exp, scale=NEG)
        rs = sb.tile([128, rcols], bf16, tag="rs", bufs=2)
        nc.vector.tensor_tensor(
            out=rs[:, HF:rcols], in0=ex1s[:], in1=ex2s[:], op=OP.max
        )
        nc.vector.tensor_tensor(
            out=rs[:, 0:HF].rearrange("p (h f) -> p h f", h=H),
            in0=own[:, 0:HF].rearrange("p (h f) -> p h f", h=H),
            in1=rs[:, HF:rcols][:, :, None].to_broadcast([128, H, F]),
            op=OP.mult,
        )
        # accumulate [num | den]: self-loop first, then the gathered sub-tiles
        psa = ps.tile([128, rcols], f32, tag="psa", bufs=2)
        nc.tensor.matmul(
            out=psa[:], lhsT=ident_t[:], rhs=rs[:], start=True, stop=False
        )
        for j in range(NSUB):
            nc.tensor.matmul(
                out=psa[:],
                lhsT=U[:, j, :],
                rhs=gw[:, j, :],
                start=False,
                stop=(j == NSUB - 1),
            )
        # normalize + affine + activation
        rden = sb.tile([128, H], f32, tag="rden", bufs=2)
        nc.vector.reciprocal_approx_fast(out=rden[:], in_=psa[:, HF:rcols])
        o1 = sb.tile([128, HF], f32, tag="o1", bufs=2)
        nc.vector.tensor_tensor(
            out=o1[:].rearrange("p (h f) -> p h f", h=H),
            in0=psa[:, 0:HF].rearrange("p (h f) -> p h f", h=H),
            in1=rden[:].to_broadcast([128, H, F]),
            op=OP.mult,
        )
        o3 = sb.tile([128, HF], f32, tag="o3", bufs=2)
        nc.vector.tensor_tensor(out=o3[:], in0=o1[:], in1=bn_c_t[:], op=OP.add)
        if final:
            outt = sb.tile([128, HF], f32, tag="outt", bufs=2)
            nc.scalar.activation(out=outt[:], in_=o3[:], func=AF.Sigmoid)
            nc.sync.dma_start(out=h_out[b * 128 : (b + 1) * 128, :], in_=outt[:])
        else:
            # elu(x) = max(x, exp(min(x, 0)) - 1)
            e1 = sb.tile([128, HF], f32, tag="e1", bufs=2)
            nc.vector.tensor_scalar_min(out=e1[:], in0=o3[:], scalar1=0.0)
            e2 = sb.tile([128, HF], f32, tag="e2", bufs=2)
            nc.scalar.activation(out=e2[:], in_=e1[:], func=AF.Exp)
            hb = sb.tile([128, HF], bf16, tag="hb", bufs=2)
            nc.vector.scalar_tensor_tensor(
                out=hb[:],
                in0=e2[:],
                scalar=-1.0,
                in1=o3[:],
                op0=OP.add,
                op1=OP.max,
            )
            nc.sync.dma_start(out=h_out[b * 128 : (b + 1) * 128, :], in_=hb[:])


def _build_program(NLO, NHI):
    nc = bacc.Bacc(
        "TRN2",
        target_bir_lowering=False,
        debug=False,
        num_devices=NCORES,
        num_swdge_queues=4,
    )
    HD = HEADS * HID
    NSUB = NLO + NHI
    ML = NLO * 8 + NHI * 8 + NSUB

    # --- inputs ---
    t_x = nc.dram_tensor("x_shard", [SHARD, DIN], bf16, kind="ExternalInput")
    t_w1 = nc.dram_tensor("W1p", [DIN, USED12], bf16, kind="ExternalInput")
    t_w2 = nc.dram_tensor("W2p", [HD, USED12], bf16, kind="ExternalInput")
    t_w3 = nc.dram_tensor("W3p", [HD, USED3], bf16, kind="ExternalInput")
    t_c1 = nc.dram_tensor("bn_c1", [128, HD], f32, kind="ExternalInput")
    t_c2 = nc.dram_tensor("bn_c2", [128, HD], f32, kind="ExternalInput")
    t_c3 = nc.dram_tensor("bn_c3", [128, DOUT], f32, kind="ExternalInput")
    t_iota = nc.dram_tensor("iota_bf", [128, 128], bf16, kind="ExternalInput")
    t_meta = nc.dram_tensor("meta", [NBLK, 128, ML], i16, kind="ExternalInput")
    t_counts = nc.dram_tensor("counts", [1, NBLK * 2], i32, kind="ExternalInput")
    t_out = nc.dram_tensor("out_shard", [SHARD, DOUT], f32, kind="ExternalOutput")

    with tile.TileContext(nc) as tc:
        with (
            tc.tile_pool(name="sb", bufs=2) as sb,
            tc.tile_pool(name="ps", bufs=2, space="PSUM") as ps,
            tc.tile_pool(name="dram", bufs=1, space="DRAM") as dr,
        ):
            # DRAM intermediates (pool tiles so Tile tracks dependencies)
            tb1_shard = dr.tile([SHARD, DROW12], bf16, name="tb1_shard")
            tb1_full = dr.tile([NPAD, DROW12], bf16, addr_space="Shared", name="tb1_full")
            tb2_shard = dr.tile([SHARD, DROW12], bf16, name="tb2_shard")
            tb2_full = dr.tile([NPAD, DROW12], bf16, addr_space="Shared", name="tb2_full")
            tb3_shard = dr.tile([SHARD, DROW3], bf16, name="tb3_shard")
            tb3_full = dr.tile([NPAD, DROW3], bf16, addr_space="Shared", name="tb3_full")
            own1 = dr.tile([SHARD, USED12], bf16, name="own1")
            own2 = dr.tile([SHARD, USED12], bf16, name="own2")
            own3 = dr.tile([SHARD, USED3], bf16, name="own3")
            h2_own = dr.tile([SHARD, HD], bf16, name="h2_own")
            h3_own = dr.tile([SHARD, HD], bf16, name="h3_own")

            # constants
            iota_t = sb.tile([128, 128], bf16, tag="iota", bufs=1)
            nc.sync.dma_start(out=iota_t[:], in_=t_iota[:])
            ident_t = sb.tile([128, 128], bf16, tag="ident", bufs=1)
            make_identity(nc, ident_t[:])
            counts_t = sb.tile([1, NBLK * 2], i32, tag="counts", bufs=1)
            nc.sync.dma_start(out=counts_t[:], in_=t_counts[:])
            w1t = [sb.tile([128, USED12], bf16, tag="w1", bufs=1, name="w1t0")]
            nc.sync.dma_start(out=w1t[0][:], in_=t_w1[:])
            w2t = [sb.tile([128, USED12], bf16, tag=f"w2_{k}", bufs=1, name=f"w2t{k}") for k in range(2)]
            for k in range(2):
                nc.sync.dma_start(out=w2t[k][:], in_=t_w2[k * 128 : (k + 1) * 128, :])
            w3t = [sb.tile([128, USED3], bf16, tag=f"w3_{k}", bufs=1, name=f"w3t{k}") for k in range(2)]
            for k in range(2):
                nc.sync.dma_start(out=w3t[k][:], in_=t_w3[k * 128 : (k + 1) * 128, :])
            bn = {}
            for nm, t, w in (("c1", t_c1, HD), ("c2", t_c2, HD), ("c3", t_c3, DOUT)):
                bt = sb.tile([128, w], f32, tag=f"bn{nm}", bufs=1, name=f"bn{nm}")
                nc.sync.dma_start(out=bt[:], in_=t[:])
                bn[nm] = bt

            rg = [list(range(NCORES))]

            # ---- layer 1 ----
            _dense_phase(nc, tc, sb, ps, t_x, w1t, tb1_shard, own1, USED12, DIN)
            nc.gpsimd.collective_compute(
                "AllGather", OP.bypass, replica_groups=rg,
                ins=[tb1_shard[:]], outs=[tb1_full[:]],
            )
            _edge_phase(
                nc, tc, sb, ps, tb1_full, own1, t_meta, t_counts, counts_t,
                iota_t, ident_t, HEADS, HID, DROW12, USED12, bn["c1"],
                h2_own, False, NLO, NHI,
            )
            # ---- layer 2 ----
            _dense_phase(nc, tc, sb, ps, h2_own, w2t, tb2_shard, own2, USED12, HD)
            nc.gpsimd.collective_compute(
                "AllGather", OP.bypass, replica_groups=rg,
                ins=[tb2_shard[:]], outs=[tb2_full[:]],
            )
            _edge_phase(
                nc, tc, sb, ps, tb2_full, own2, t_meta, t_counts, counts_t,
                iota_t, ident_t, HEADS, HID, DROW12, USED12, bn["c2"],
                h3_own, False, NLO, NHI,
            )
            # ---- layer 3 ----
            _dense_phase(nc, tc, sb, ps, h3_own, w3t, tb3_shard, own3, USED3, HD)
            nc.gpsimd.collective_compute(
                "AllGather", OP.bypass, replica_groups=rg,
                ins=[tb3_shard[:]], outs=[tb3_full[:]],
            )
            _edge_phase(
                nc, tc, sb, ps, tb3_full, own3, t_meta, t_counts, counts_t,
                iota_t, ident_t, 1, DOUT, DROW3, USED3, bn["c3"],
                t_out, True, NLO, NHI,
            )

    nc.compile()
    return nc


_CACHED = {}


def kernel(**inputs):
    x = np.asarray(inputs["x"], np.float32)
    edge_src = np.asarray(inputs["edge_src"], np.int32)
    edge_dst = np.asarray(inputs["edge_dst"], np.int32)

    xp = np.zeros((NPAD, DIN), np.float32)
    xp[:N] = x
    xb = xp.astype(bfnp)

    def aff(g, v, b, m, be):
        a = np.asarray(g, np.float32) / np.sqrt(np.asarray(v, np.float32) + EPS)
        c = (np.asarray(b, np.float32) - np.asarray(m, np.float32)) * a + np.asarray(
            be, np.float32
        )
        return a, c

    a1, c1 = aff(inputs["g1"], inputs["v1"], inputs["b1"], inputs["m1"], inputs["be1"])
    a2, c2 = aff(inputs["g2"], inputs["v2"], inputs["b2"], inputs["m2"], inputs["be2"])
    a3 = np.ones(DOUT, np.float32)
    c3 = np.asarray(inputs["b3"], np.float32)

    W1p = _build_Wp(
        np.asarray(inputs["W1"], np.float32),
        np.asarray(inputs["as1"], np.float32),
        np.asarray(inputs["ad1"], np.float32),
        a1,
    )
    W2p = _build_Wp(
        np.asarray(inputs["W2"], np.float32),
        np.asarray(inputs["as2"], np.float32),
        np.asarray(inputs["ad2"], np.float32),
        a2,
    )
    W3p = _build_Wp(
        np.asarray(inputs["W3"], np.float32),
        np.asarray(inputs["as3"], np.float32),
        np.asarray(inputs["ad3"], np.float32),
        a3,
    )

    edata, NLO, NHI = _prep_edges(edge_src, edge_dst)
    iota = np.tile(np.arange(128, dtype=np.float32), (128, 1)).astype(bfnp)

    key = (NLO, NHI)
    if _CACHED.get("key") != key:
        _CACHED["nc"] = _build_program(NLO, NHI)
        _CACHED["key"] = key
    nc = _CACHED["nc"]

    def bcast(v):
        return np.tile(np.asarray(v, np.float32), (128, 1))

    in_maps = []
    for c in range(NCORES):
        in_maps.append(
            {
                "x_shard": xb[c * SHARD : (c + 1) * SHARD],
                "W1p": W1p,
                "W2p": W2p,
                "W3p": W3p,
                "bn_c1": bcast(c1),
                "bn_c2": bcast(c2),
                "bn_c3": bcast(c3),
                "iota_bf": iota,
                "meta": edata[c]["meta"],
                "counts": edata[c]["counts"].reshape(1, -1),
            }
        )

    trace = bool(os.environ.get("GAT_TRACE"))
    res = run_bass_kernel_spmd(
        nc, in_maps, core_ids=list(range(NCORES)), trace=trace
    )
    if trace and res.exec_time_ns:
        print(f"HW exec time: {res.exec_time_ns} ns")
    out = np.concatenate([res.results[c]["out_shard"] for c in range(NCORES)], axis=0)
    return np.ascontiguousarray(out[:N]).astype(np.float32)


# revision 37
# speedup vs baseline: 1.6534x; 1.0453x over previous
"""GAT (3-layer, PyG GATConv semantics) on 8 Trainium2 NeuronCores.

Strategy (dst-node sharding):
- Nodes padded to 50176 = 8 * 6272; core c owns dst rows [c*6272, (c+1)*6272).
- Per layer:
  dense (sharded): table_shard = h_shard @ [W*bn_a | W@a_src | W@a_dst] in bf16
      (bn scale folded into W's feature columns on the host), written both at
      a 256B-multiple pitch (for dma_gather) and contiguous (for local reads).
  AllGather the padded table so every core can fetch arbitrary src rows.
  edge phase: edges bucketed by dst into 128-slot blocks. Self-loops are NOT
      gathered: their contribution comes from the local contiguous table via
      an identity-lhsT matmul. Remaining edges are packed lo/hi (int16 index
      range split), padded with trailing -1 indices which the gather ucode
      SKIPS -- the true per-block descriptor count is passed at runtime via
      value_load, so padding costs nothing on the DMA engines.
      Per block: batched dma_gather of src rows, one-hot selection matrices
      via is_equal(iota, dst_local), segment softmax without max-subtraction
      (logits are O(0.3)), ex-weighted features + ex accumulated into PSUM
      with one matmul per 128-edge sub-tile, then normalize / bn / activation
      fused on DVE+ACT (elu via one scalar_tensor_tensor).
- d[dst] per edge is produced on-chip: transpose the one-hot with the PE and
  multiply against the block's d rows.
"""
import os
import sys
import types

sys.path.insert(0, "/opt/trn_rl_repo")

import numpy as np
import ml_dtypes


def _install_ntff_shim():
    """Provide antenv.axon_hooks so run_bass_kernel_spmd(trace=True) works."""
    try:
        import antenv

        if "antenv.axon_hooks" in sys.modules:
            return
        mod = types.ModuleType("antenv.axon_hooks")
        mod._hook = None
        mod.set_axon_ntff_profile_hook = lambda h: setattr(mod, "_hook", h)
        mod.get_axon_ntff_profile_hook = lambda: mod._hook
        sys.modules["antenv.axon_hooks"] = mod
        antenv.axon_hooks = mod
        from trn_agent_boot.trn_boot import _ntff_profile_via_ctypes

        hook = _ntff_profile_via_ctypes("/opt/axon/libaxon_pjrt.so")
        if hook is not None:
            mod.set_axon_ntff_profile_hook(hook)
    except Exception:
        pass


_install_ntff_shim()

import concourse.bass as bass
import concourse.bacc as bacc
import concourse.mybir as mybir
import concourse.tile as tile
from concourse.bass_utils import run_bass_kernel_spmd
from concourse.masks import make_identity

bfnp = ml_dtypes.bfloat16
f32 = mybir.dt.float32
bf16 = mybir.dt.bfloat16
i16 = mybir.dt.int16
i32 = mybir.dt.int32
AF = mybir.ActivationFunctionType
OP = mybir.AluOpType

N, E = 50000, 800000
DIN, HID, HEADS, DOUT = 128, 32, 8, 16
NEG = 0.2
EPS = 1e-5

NCORES = 8
SHARD = 6272
NPAD = NCORES * SHARD  # 50176
NBLK = SHARD // 128  # 49
LOCUT = 32768
HI_BASE = NPAD - LOCUT  # 17408

# table row pitches (bf16 cols; byte pitch must be a multiple of 256)
DROW12, USED12 = 384, 272  # [xw(256) | s(8) | d(8) | pad]
DROW3, USED3 = 128, 18  # [xw(16) | s(1) | d(1) | pad]
PAD_DSTL = 200.0  # one-hot miss marker for padding edges
GBUFS = 4
SINGLE_PACKET = False

# The full table is laid out chunk-major: chunk 0 holds every core's shard
# rows [0, CH0), chunk 1 the rest. Each chunk is then a CONTIGUOUS AllGather
# destination, and chunk 0's collective overlaps chunk 1's dense phase.
CH0_BLKS = 25
CH0 = CH0_BLKS * 128  # 3200
CH1 = SHARD - CH0  # 3072
POS0 = NCORES * CH0  # 25600


def _pos(g):
    """global node id -> chunk-major position in the gathered table."""
    c, r = g // SHARD, g % SHARD
    return np.where(r < CH0, c * CH0 + r, POS0 + c * CH1 + (r - CH0))


def _build_Wp(W, a_s, a_d, bn_a):
    H, F = a_s.shape
    Ws = np.stack([W[:, h * F : (h + 1) * F] @ a_s[h] for h in range(H)], axis=1)
    Wd = np.stack([W[:, h * F : (h + 1) * F] @ a_d[h] for h in range(H)], axis=1)
    return np.concatenate([W * bn_a[None, :], Ws, Wd], axis=1).astype(bfnp)


def _wrap_idx(flat):
    """int16 list -> [128, len/16] wrapped in 16 partitions, replicated x8."""
    n = len(flat)
    assert n % 16 == 0
    w = flat.reshape(n // 16, 16).T  # [16, n/16]
    return np.tile(w, (8, 1)).astype(np.int16)


def _prep_edges(edge_src, edge_dst):
    """Bucket non-self-loop edges by (core, block); returns per-core packed
    meta arrays (idx_lo | idx_hi | dstl) plus per-block valid counts, and the
    (NLO, NHI) sub-tile split chosen from the data."""
    src = _pos(edge_src.astype(np.int64))
    dst = edge_dst.astype(np.int64)
    core = dst // SHARD
    blk = (dst % SHARD) // 128

    # per-(core, block) stats to pick NLO/NHI
    nlo_need, nhi_need = 0, 0
    per_block = {}
    for c in range(NCORES):
        m = core == c
        for b in range(NBLK):
            mb = m & (blk == b)
            s_all, d_all = src[mb], dst[mb]
            order = np.argsort(s_all, kind="stable")
            per_block[(c, b)] = (s_all[order], d_all[order])

    # lo gather reads table chunk 0 (positions < POS0), hi reads chunk 1.
    def feasible(nlo, nhi):
        for (c, b), (s_all, _) in per_block.items():
            if int((s_all < POS0).sum()) > nlo * 128:
                return False
            if int((s_all >= POS0).sum()) > nhi * 128:
                return False
        return True

    nsub = None
    for tot in range(15, 22):
        ok = [
            (nlo, tot - nlo)
            for nlo in range(tot // 2, tot)
            if feasible(nlo, tot - nlo)
        ]
        if ok:
            nsub = tot
            nlo_need, nhi_need = ok[0]
            break
    assert nsub is not None, "no feasible (NLO, NHI) split"
    NLO, NHI = nlo_need, nhi_need
    NSUB = NLO + NHI
    ML = NLO * 8 + NHI * 8 + NSUB  # meta cols (int16)

    out = []
    for c in range(NCORES):
        meta = np.zeros((NBLK, 128, ML), np.int16)
        counts = np.zeros((NBLK, 2), np.int32)
        for b in range(NBLK):
            s_all, d_all = per_block[(c, b)]
            ncut = int((s_all < POS0).sum())
            lo_s, lo_d = s_all[:ncut], d_all[:ncut]
            hi_s, hi_d = s_all[ncut:] - POS0, d_all[ncut:]
            dstl = np.full((NSUB * 128,), PAD_DSTL, np.float32)
            for half, (ss, dd, cap, off) in enumerate(
                ((lo_s, lo_d, NLO, 0), (hi_s, hi_d, NHI, NLO))
            ):
                n = len(ss)
                assert n <= cap * 128, (c, b, half, n)
                sp = np.full(cap * 128, -1, np.int16)
                sp[:n] = ss.astype(np.int16)
                dl = dstl[off * 128 : (off + cap) * 128]
                dl[:n] = (dd - c * SHARD - b * 128).astype(np.float32)
                if n == 0:  # keep the gather ucode's count >= 1
                    sp[0], n = 0, 1
                counts[b, half] = n
                w = _wrap_idx(sp)
                if half == 0:
                    meta[b, :, 0 : NLO * 8] = w
                else:
                    meta[b, :, NLO * 8 : NLO * 8 + NHI * 8] = w
            # gathered row i lands at [p=i%128, j=i//128]
            dst_slot = dstl.reshape(NSUB, 128).T.astype(bfnp)  # [128, NSUB]
            meta[b, :, NLO * 8 + NHI * 8 :] = dst_slot.view(np.int16)
        out.append({"meta": meta, "counts": counts.reshape(-1)})
    return out, NLO, NHI


def _dense_phase(nc, tc, sb, ps, h_in, w_tiles, own_tab, tb_c0, tb_c1, used, h_cols):
    """own_tab = h_in @ W' (contiguous), also written into the padded
    chunk-major table halves tb_c0/tb_c1 that feed the AllGathers. h_in is a
    DRAM [SHARD, h_cols] bf16 tensor (row-major). Transpose-loads and table
    writes are batched 4 blocks at a time to keep the HWDGE queue short."""
    nk = h_cols // 128
    BB = 4  # blocks per batch; batches never straddle the chunk boundary
    t = 0
    while t < NBLK:
        lim = CH0_BLKS if t < CH0_BLKS else NBLK
        nb = min(BB, lim - t)
        hts = []
        for k in range(nk):
            ht = sb.tile([128, nb * 128], bf16, tag="ht", bufs=2, name=f"ht{k}")
            nc.sync.dma_start(
                out=ht[:],
                in_=h_in[t * 128 : (t + nb) * 128, k * 128 : (k + 1) * 128],
                transpose=True,
            )
            hts.append(ht)
        rows = sb.tile([128, nb, used], bf16, tag="drow", bufs=2)
        for j in range(nb):
            psd = ps.tile([128, used], f32, tag="psd", bufs=2)
            for k in range(nk):
                nc.tensor.matmul(
                    out=psd[:],
                    lhsT=hts[k][:, j * 128 : (j + 1) * 128],
                    rhs=w_tiles[k][:],
                    start=(k == 0),
                    stop=(k == nk - 1),
                )
            nc.scalar.activation(out=rows[:, j, :], in_=psd[:], func=AF.Copy)
        nc.sync.dma_start(
            out=own_tab[t * 128 : (t + nb) * 128, :].rearrange(
                "(j p) c -> p j c", j=nb
            ),
            in_=rows[:, 0:nb, :],
        )
        if t < CH0_BLKS:
            tb_c, r0 = tb_c0, t * 128
        else:
            tb_c, r0 = tb_c1, t * 128 - CH0
        nc.sync.dma_start(
            out=tb_c[r0 : r0 + nb * 128, 0:used].rearrange(
                "(j p) c -> p j c", j=nb
            ),
            in_=rows[:, 0:nb, :],
        )
        t += nb


def _edge_phase(
    nc,
    tc,
    sb,
    ps,
    table_c0,
    table_c1,
    own_tab,
    t_meta,
    t_counts,
    counts_t,
    iota_t,
    ident_t,
    H,
    F,
    drow,
    used,
    bn_c_t,
    h_out,
    final,
    NLO,
    NHI,
):
    NSUB = NLO + NHI
    HF = H * F
    rcols = HF + H  # matmul rhs cols: [gw | ex]
    ML = NLO * 8 + NHI * 8 + NSUB

    r_lo = nc.gpsimd.alloc_register(f"cnt_lo_{nc.next_id()}")
    r_hi = nc.gpsimd.alloc_register(f"cnt_hi_{nc.next_id()}")
    for b in range(NBLK):
        qlo, qhi = (2 * b) % 4, (2 * b + 1) % 4
        meta = sb.tile([128, ML], i16, tag="meta", bufs=4)
        nc.sync.dma_start(out=meta[:], in_=t_meta[b])
        own = sb.tile([128, used], bf16, tag="own", bufs=3)
        nc.sync.dma_start(out=own[:], in_=own_tab[b * 128 : (b + 1) * 128, :])
        dstl = meta[:, NLO * 8 + NHI * 8 :].bitcast(bf16)

        nc.gpsimd.reg_load(r_lo, counts_t[0:1, 2 * b : 2 * b + 1])
        nc.gpsimd.reg_load(r_hi, counts_t[0:1, 2 * b + 1 : 2 * b + 2])
        cnt_lo, cnt_hi = r_lo, r_hi

        # gather slots skipped by the runtime descriptor count would otherwise
        # expose stale SBUF to the alpha/gw reads (NaN risk): zero every
        # column the block reads before gathering over it.
        G = sb.tile([128, NSUB, drow], bf16, tag="G", bufs=GBUFS)
        nc.vector.memset(G[:, :, 0 : HF + H], 0.0)
        nc.gpsimd.dma_gather(
            out_ap=G[:, 0:NLO, :],
            in_ap=table_c0[:],
            idxs_ap=meta[:16, 0 : NLO * 8],
            num_idxs=NLO * 128,
            num_idxs_reg=cnt_lo,
            elem_size=drow,
            single_packet=SINGLE_PACKET,
            queue_num=qlo,
        )
        nc.gpsimd.dma_gather(
            out_ap=G[:, NLO:NSUB, :],
            in_ap=table_c1[:],
            idxs_ap=meta[:16, NLO * 8 : NLO * 8 + NHI * 8],
            num_idxs=NHI * 128,
            num_idxs_reg=cnt_hi,
            elem_size=drow,
            single_packet=SINGLE_PACKET,
            queue_num=qhi,
        )

        # one-hot selection matrices, U[e, j, slot] = (dst_local[e, j] == slot)
        U = sb.tile([128, NSUB, 128], bf16, tag="U", bufs=2)
        nc.vector.tensor_tensor(
            out=U[:],
            in0=iota_t[:, None, :].to_broadcast([128, NSUB, 128]),
            in1=dstl[:, :, None].to_broadcast([128, NSUB, 128]),
            op=OP.is_equal,
        )
        # transposed one-hots (for d expansion), via PE transpose in groups of 4
        Ut = sb.tile([128, NSUB, 128], bf16, tag="Ut", bufs=2)
        for g in range(0, NSUB, 4):
            n = min(4, NSUB - g)
            pst = ps.tile([128, 512], bf16, tag="pst", bufs=2)
            for k in range(n):
                nc.tensor.transpose(
                    out=pst[:, k * 128 : (k + 1) * 128],
                    in_=U[:, g + k, :],
                    identity=ident_t[:],
                )
            nc.vector.tensor_copy(
                out=Ut[:, g : g + n, :],
                in_=pst[:, : n * 128].rearrange("p (j e) -> p j e", j=n),
            )
        # d per edge: d_pe[e, h] = d_blk[dst_local[e], h]
        dblk = own[:, HF + H : HF + 2 * H]
        psd = ps.tile([128, NSUB * H], f32, tag="psdpe", bufs=2)
        for j in range(NSUB):
            nc.tensor.matmul(
                out=psd[:, j * H : (j + 1) * H],
                lhsT=Ut[:, j, :],
                rhs=dblk,
                start=True,
                stop=True,
            )
        # ex = exp(leaky_relu(s_src + d_dst)) == max(exp(x), exp(0.2 x))
        alpha = sb.tile([128, NSUB * H], f32, tag="alpha", bufs=2)
        nc.vector.tensor_tensor(
            out=alpha[:].rearrange("p (j h) -> p j h", j=NSUB),
            in0=G[:, :, HF : HF + H],
            in1=psd[:].rearrange("p (j h) -> p j h", j=NSUB),
            op=OP.add,
        )
        ex1 = sb.tile([128, NSUB * H], bf16, tag="ex1", bufs=2)
        nc.scalar.activation(out=ex1[:], in_=alpha[:], func=AF.Exp)
        ex2 = sb.tile([128, NSUB * H], bf16, tag="ex2", bufs=2)
        nc.scalar.activation(out=ex2[:], in_=alpha[:], func=AF.Exp, scale=NEG)
        # rhs tile for the psa accumulation: [xw*ex | ex]; the max lands
        # directly in the ex columns and the multiply re-reads them.
        gw = sb.tile([128, NSUB, rcols], bf16, tag="gw", bufs=2)
        nc.vector.tensor_tensor(
            out=gw[:, :, HF:rcols],
            in0=ex1[:].rearrange("p (j h) -> p j h", j=NSUB),
            in1=ex2[:].rearrange("p (j h) -> p j h", j=NSUB),
            op=OP.max,
        )
        nc.vector.tensor_tensor(
            out=gw[:, :, 0:HF].rearrange("p j (h f) -> p j h f", h=H),
            in0=G[:, :, 0:HF].rearrange("p j (h f) -> p j h f", h=H),
            in1=gw[:, :, HF:rcols][:, :, :, None].to_broadcast(
                [128, NSUB, H, F]
            ),
            op=OP.mult,
        )
        # self-loop contribution from the local contiguous table
        alph_s = sb.tile([128, H], f32, tag="alphs", bufs=2)
        nc.vector.tensor_tensor(
            out=alph_s[:],
            in0=own[:, HF : HF + H],
            in1=own[:, HF + H : HF + 2 * H],
            op=OP.add,
        )
        ex1s = sb.tile([128, H], bf16, tag="ex1s", bufs=2)
        nc.scalar.activation(out=ex1s[:], in_=alph_s[:], func=AF.Exp)
        ex2s = sb.tile([128, H], bf16, tag="ex2s", bufs=2)
        nc.scalar.activation(out=ex2s[:], in_=alph_s[:], func=AF.Exp, scale=NEG)
        rs = sb.tile([128, rcols], bf16, tag="rs", bufs=2)
        nc.vector.tensor_tensor(
            out=rs[:, HF:rcols], in0=ex1s[:], in1=ex2s[:], op=OP.max
        )
        nc.vector.tensor_tensor(
            out=rs[:, 0:HF].rearrange("p (h f) -> p h f", h=H),
            in0=own[:, 0:HF].rearrange("p (h f) -> p h f", h=H),
            in1=rs[:, HF:rcols][:, :, None].to_broadcast([128, H, F]),
            op=OP.mult,
        )
        # accumulate [num | den]: self-loop first, then the gathered sub-tiles
        psa = ps.tile([128, rcols], f32, tag="psa", bufs=2)
        nc.tensor.matmul(
            out=psa[:], lhsT=ident_t[:], rhs=rs[:], start=True, stop=False
        )
        for j in range(NSUB):
            nc.tensor.matmul(
                out=psa[:],
                lhsT=U[:, j, :],
                rhs=gw[:, j, :],
                start=False,
                stop=(j == NSUB - 1),
            )
        # normalize + affine + activation
        rden = sb.tile([128, H], f32, tag="rden", bufs=2)
        nc.vector.reciprocal_approx_fast(out=rden[:], in_=psa[:, HF:rcols])
        o1 = sb.tile([128, HF], f32, tag="o1", bufs=2)
        nc.vector.tensor_tensor(
            out=o1[:].rearrange("p (h f) -> p h f", h=H),
            in0=psa[:, 0:HF].rearrange("p (h f) -> p h f", h=H),
            in1=rden[:].to_broadcast([128, H, F]),
            op=OP.mult,
        )
        o3 = sb.tile([128, HF], f32, tag="o3", bufs=2)
        nc.vector.tensor_tensor(out=o3[:], in0=o1[:], in1=bn_c_t[:], op=OP.add)
        if final:
            outt = sb.tile([128, HF], f32, tag="outt", bufs=2)
            nc.scalar.activation(out=outt[:], in_=o3[:], func=AF.Sigmoid)
            nc.sync.dma_start(out=h_out[b * 128 : (b + 1) * 128, :], in_=outt[:])
        else:
            # elu(x) = max(x, exp(min(x, 0)) - 1)
            e1 = sb.tile([128, HF], f32, tag="e1", bufs=2)
            nc.vector.tensor_scalar_min(out=e1[:], in0=o3[:], scalar1=0.0)
            e2 = sb.tile([128, HF], f32, tag="e2", bufs=2)
            nc.scalar.activation(out=e2[:], in_=e1[:], func=AF.Exp)
            hb = sb.tile([128, HF], bf16, tag="hb", bufs=2)
            nc.vector.scalar_tensor_tensor(
                out=hb[:],
                in0=e2[:],
                scalar=-1.0,
                in1=o3[:],
                op0=OP.add,
                op1=OP.max,
            )
            nc.sync.dma_start(out=h_out[b * 128 : (b + 1) * 128, :], in_=hb[:])


def _build_program(NLO, NHI):
    nc = bacc.Bacc(
        "TRN2",
        target_bir_lowering=False,
        debug=False,
        num_devices=NCORES,
        num_swdge_queues=4,
    )
    HD = HEADS * HID
    NSUB = NLO + NHI
    ML = NLO * 8 + NHI * 8 + NSUB

    # --- inputs ---
    t_x = nc.dram_tensor("x_shard", [SHARD, DIN], bf16, kind="ExternalInput")
    t_w1 = nc.dram_tensor("W1p", [DIN, USED12], bf16, kind="ExternalInput")
    t_w2 = nc.dram_tensor("W2p", [HD, USED12], bf16, kind="ExternalInput")
    t_w3 = nc.dram_tensor("W3p", [HD, USED3], bf16, kind="ExternalInput")
    t_c1 = nc.dram_tensor("bn_c1", [128, HD], f32, kind="ExternalInput")
    t_c2 = nc.dram_tensor("bn_c2", [128, HD], f32, kind="ExternalInput")
    t_c3 = nc.dram_tensor("bn_c3", [128, DOUT], f32, kind="ExternalInput")
    t_iota = nc.dram_tensor("iota_bf", [128, 128], bf16, kind="ExternalInput")
    t_meta = nc.dram_tensor("meta", [NBLK, 128, ML], i16, kind="ExternalInput")
    t_counts = nc.dram_tensor("counts", [1, NBLK * 2], i32, kind="ExternalInput")
    t_out = nc.dram_tensor("out_shard", [SHARD, DOUT], f32, kind="ExternalOutput")

    with tile.TileContext(nc) as tc:
        with (
            tc.tile_pool(name="sb", bufs=2) as sb,
            tc.tile_pool(name="ps", bufs=2, space="PSUM") as ps,
            tc.tile_pool(name="dram", bufs=1, space="DRAM") as dr,
        ):
            # DRAM intermediates (pool tiles so Tile tracks dependencies)
            tbf = {
                (lyr, k): dr.tile(
                    [NCORES * (CH0 if k == 0 else CH1), DROW12 if lyr < 3 else DROW3],
                    bf16,
                    addr_space="Shared",
                    name=f"tb{lyr}_full{k}",
                )
                for lyr in (1, 2, 3)
                for k in (0, 1)
            }
            tbc = {
                (lyr, k): dr.tile(
                    [CH0 if k == 0 else CH1, DROW12 if lyr < 3 else DROW3],
                    bf16,
                    name=f"tb{lyr}_c{k}",
                )
                for lyr in (1, 2, 3)
                for k in (0, 1)
            }
            own1 = dr.tile([SHARD, USED12], bf16, name="own1")
            own2 = dr.tile([SHARD, USED12], bf16, name="own2")
            own3 = dr.tile([SHARD, USED3], bf16, name="own3")
            h2_own = dr.tile([SHARD, HD], bf16, name="h2_own")
            h3_own = dr.tile([SHARD, HD], bf16, name="h3_own")

            # constants
            iota_t = sb.tile([128, 128], bf16, tag="iota", bufs=1)
            nc.sync.dma_start(out=iota_t[:], in_=t_iota[:])
            ident_t = sb.tile([128, 128], bf16, tag="ident", bufs=1)
            make_identity(nc, ident_t[:])
            counts_t = sb.tile([1, NBLK * 2], i32, tag="counts", bufs=1)
            nc.sync.dma_start(out=counts_t[:], in_=t_counts[:])
            w1t = [sb.tile([128, USED12], bf16, tag="w1", bufs=1, name="w1t0")]
            nc.sync.dma_start(out=w1t[0][:], in_=t_w1[:])
            w2t = [sb.tile([128, USED12], bf16, tag=f"w2_{k}", bufs=1, name=f"w2t{k}") for k in range(2)]
            for k in range(2):
                nc.sync.dma_start(out=w2t[k][:], in_=t_w2[k * 128 : (k + 1) * 128, :])
            w3t = [sb.tile([128, USED3], bf16, tag=f"w3_{k}", bufs=1, name=f"w3t{k}") for k in range(2)]
            for k in range(2):
                nc.sync.dma_start(out=w3t[k][:], in_=t_w3[k * 128 : (k + 1) * 128, :])
            bn = {}
            for nm, t, w in (("c1", t_c1, HD), ("c2", t_c2, HD), ("c3", t_c3, DOUT)):
                bt = sb.tile([128, w], f32, tag=f"bn{nm}", bufs=1, name=f"bn{nm}")
                nc.sync.dma_start(out=bt[:], in_=t[:])
                bn[nm] = bt

            rg = [list(range(NCORES))]

            def ag_chunks(lyr):
                for k in (0, 1):
                    nc.gpsimd.collective_compute(
                        "AllGather", OP.bypass, replica_groups=rg,
                        ins=[tbc[(lyr, k)][:]], outs=[tbf[(lyr, k)][:]],
                    )

            # ---- layer 1 ----
            _dense_phase(
                nc, tc, sb, ps, t_x, w1t, own1, tbc[(1, 0)], tbc[(1, 1)],
                USED12, DIN,
            )
            ag_chunks(1)
            _edge_phase(
                nc, tc, sb, ps, tbf[(1, 0)], tbf[(1, 1)], own1, t_meta, t_counts, counts_t,
                iota_t, ident_t, HEADS, HID, DROW12, USED12, bn["c1"],
                h2_own, False, NLO, NHI,
            )
            # ---- layer 2 ----
            _dense_phase(
                nc, tc, sb, ps, h2_own, w2t, own2, tbc[(2, 0)], tbc[(2, 1)],
                USED12, HD,
            )
            ag_chunks(2)
            _edge_phase(
                nc, tc, sb, ps, tbf[(2, 0)], tbf[(2, 1)], own2, t_meta, t_counts, counts_t,
                iota_t, ident_t, HEADS, HID, DROW12, USED12, bn["c2"],
                h3_own, False, NLO, NHI,
            )
            # ---- layer 3 ----
            _dense_phase(
                nc, tc, sb, ps, h3_own, w3t, own3, tbc[(3, 0)], tbc[(3, 1)],
                USED3, HD,
            )
            ag_chunks(3)
            _edge_phase(
                nc, tc, sb, ps, tbf[(3, 0)], tbf[(3, 1)], own3, t_meta, t_counts, counts_t,
                iota_t, ident_t, 1, DOUT, DROW3, USED3, bn["c3"],
                t_out, True, NLO, NHI,
            )

    nc.compile()
    return nc


_CACHED = {}


def kernel(**inputs):
    x = np.asarray(inputs["x"], np.float32)
    edge_src = np.asarray(inputs["edge_src"], np.int32)
    edge_dst = np.asarray(inputs["edge_dst"], np.int32)

    xp = np.zeros((NPAD, DIN), np.float32)
    xp[:N] = x
    xb = xp.astype(bfnp)

    def aff(g, v, b, m, be):
        a = np.asarray(g, np.float32) / np.sqrt(np.asarray(v, np.float32) + EPS)
        c = (np.asarray(b, np.float32) - np.asarray(m, np.float32)) * a + np.asarray(
            be, np.float32
        )
        return a, c

    a1, c1 = aff(inputs["g1"], inputs["v1"], inputs["b1"], inputs["m1"], inputs["be1"])
    a2, c2 = aff(inputs["g2"], inputs["v2"], inputs["b2"], inputs["m2"], inputs["be2"])
    a3 = np.ones(DOUT, np.float32)
    c3 = np.asarray(inputs["b3"], np.float32)

    W1p = _build_Wp(
        np.asarray(inputs["W1"], np.float32),
        np.asarray(inputs["as1"], np.float32),
        np.asarray(inputs["ad1"], np.float32),
        a1,
    )
    W2p = _build_Wp(
        np.asarray(inputs["W2"], np.float32),
        np.asarray(inputs["as2"], np.float32),
        np.asarray(inputs["ad2"], np.float32),
        a2,
    )
    W3p = _build_Wp(
        np.asarray(inputs["W3"], np.float32),
        np.asarray(inputs["as3"], np.float32),
        np.asarray(inputs["ad3"], np.float32),
        a3,
    )

    edata, NLO, NHI = _prep_edges(edge_src, edge_dst)
    iota = np.tile(np.arange(128, dtype=np.float32), (128, 1)).astype(bfnp)

    key = (NLO, NHI)
    if _CACHED.get("key") != key:
        _CACHED["nc"] = _build_program(NLO, NHI)
        _CACHED["key"] = key
    nc = _CACHED["nc"]

    def bcast(v):
        return np.tile(np.asarray(v, np.float32), (128, 1))

    in_maps = []
    for c in range(NCORES):
        in_maps.append(
            {
                "x_shard": xb[c * SHARD : (c + 1) * SHARD],
                "W1p": W1p,
                "W2p": W2p,
                "W3p": W3p,
                "bn_c1": bcast(c1),
                "bn_c2": bcast(c2),
                "bn_c3": bcast(c3),
                "iota_bf": iota,
                "meta": edata[c]["meta"],
                "counts": edata[c]["counts"].reshape(1, -1),
            }
        )

    trace = bool(os.environ.get("GAT_TRACE"))
    res = run_bass_kernel_spmd(
        nc, in_maps, core_ids=list(range(NCORES)), trace=trace
    )
    if trace and res.exec_time_ns:
        print(f"HW exec time: {res.exec_time_ns} ns")
    out = np.concatenate([res.results[c]["out_shard"] for c in range(NCORES)], axis=0)
    return np.ascontiguousarray(out[:N]).astype(np.float32)


# revision 38
# speedup vs baseline: 2.0003x; 1.2098x over previous
"""GAT (3-layer, PyG GATConv semantics) on 8 Trainium2 NeuronCores.

Strategy (dst-node sharding):
- Nodes padded to 50176 = 8 * 6272; core c owns dst rows [c*6272, (c+1)*6272).
- Per layer:
  dense (sharded): table_shard = h_shard @ [W*bn_a | W@a_src | W@a_dst] in bf16
      (bn scale folded into W's feature columns on the host), written both at
      a 256B-multiple pitch (for dma_gather) and contiguous (for local reads).
  AllGather the padded table so every core can fetch arbitrary src rows.
  edge phase: edges bucketed by dst into 128-slot blocks. Self-loops are NOT
      gathered: their contribution comes from the local contiguous table via
      an identity-lhsT matmul. Remaining edges are packed lo/hi (int16 index
      range split), padded with trailing -1 indices which the gather ucode
      SKIPS -- the true per-block descriptor count is passed at runtime via
      value_load, so padding costs nothing on the DMA engines.
      Per block: batched dma_gather of src rows, one-hot selection matrices
      via is_equal(iota, dst_local), segment softmax without max-subtraction
      (logits are O(0.3)), ex-weighted features + ex accumulated into PSUM
      with one matmul per 128-edge sub-tile, then normalize / bn / activation
      fused on DVE+ACT (elu via one scalar_tensor_tensor).
- d[dst] per edge is produced on-chip: transpose the one-hot with the PE and
  multiply against the block's d rows.
"""
import os
import sys
import types

sys.path.insert(0, "/opt/trn_rl_repo")

import numpy as np
import ml_dtypes


def _install_ntff_shim():
    """Provide antenv.axon_hooks so run_bass_kernel_spmd(trace=True) works."""
    try:
        import antenv

        if "antenv.axon_hooks" in sys.modules:
            return
        mod = types.ModuleType("antenv.axon_hooks")
        mod._hook = None
        mod.set_axon_ntff_profile_hook = lambda h: setattr(mod, "_hook", h)
        mod.get_axon_ntff_profile_hook = lambda: mod._hook
        sys.modules["antenv.axon_hooks"] = mod
        antenv.axon_hooks = mod
        from trn_agent_boot.trn_boot import _ntff_profile_via_ctypes

        hook = _ntff_profile_via_ctypes("/opt/axon/libaxon_pjrt.so")
        if hook is not None:
            mod.set_axon_ntff_profile_hook(hook)
    except Exception:
        pass


_install_ntff_shim()

import concourse.bass as bass
import concourse.bacc as bacc
import concourse.mybir as mybir
import concourse.tile as tile
from concourse.bass_utils import run_bass_kernel_spmd
from concourse.masks import make_identity

bfnp = ml_dtypes.bfloat16
f32 = mybir.dt.float32
bf16 = mybir.dt.bfloat16
i16 = mybir.dt.int16
i32 = mybir.dt.int32
AF = mybir.ActivationFunctionType
OP = mybir.AluOpType

N, E = 50000, 800000
DIN, HID, HEADS, DOUT = 128, 32, 8, 16
NEG = 0.2
EPS = 1e-5

NCORES = 8
SHARD = 6272
NPAD = NCORES * SHARD  # 50176
NBLK = SHARD // 128  # 49
LOCUT = 32768
HI_BASE = NPAD - LOCUT  # 17408

# table row pitches (bf16 cols; byte pitch must be a multiple of 256)
DROW12, USED12 = 384, 272  # [xw(256) | s(8) | d(8) | pad]
DROW3, USED3 = 128, 18  # [xw(16) | s(1) | d(1) | pad]
PAD_DSTL = 200.0  # one-hot miss marker for padding edges
GBUFS = 4
SINGLE_PACKET = False

# The full table is laid out chunk-major: chunk 0 holds every core's shard
# rows [0, CH0), chunk 1 the rest. Each chunk is then a CONTIGUOUS AllGather
# destination, and chunk 0's collective overlaps chunk 1's dense phase.
CH0_BLKS = 25
CH0 = CH0_BLKS * 128  # 3200
CH1 = SHARD - CH0  # 3072
POS0 = NCORES * CH0  # 25600


def _pos(g):
    """global node id -> chunk-major position in the gathered table."""
    c, r = g // SHARD, g % SHARD
    return np.where(r < CH0, c * CH0 + r, POS0 + c * CH1 + (r - CH0))


def _build_Wp(W, a_s, a_d, bn_a):
    H, F = a_s.shape
    Ws = np.stack([W[:, h * F : (h + 1) * F] @ a_s[h] for h in range(H)], axis=1)
    Wd = np.stack([W[:, h * F : (h + 1) * F] @ a_d[h] for h in range(H)], axis=1)
    return np.concatenate([W * bn_a[None, :], Ws, Wd], axis=1).astype(bfnp)


def _wrap_idx(flat):
    """int16 list -> [128, len/16] wrapped in 16 partitions, replicated x8."""
    n = len(flat)
    assert n % 16 == 0
    w = flat.reshape(n // 16, 16).T  # [16, n/16]
    return np.tile(w, (8, 1)).astype(np.int16)


def _prep_edges(edge_src, edge_dst):
    """Bucket non-self-loop edges by (core, block); returns per-core packed
    meta arrays (idx_lo | idx_hi | dstl) plus per-block valid counts, and the
    (NLO, NHI) sub-tile split chosen from the data."""
    src = _pos(edge_src.astype(np.int64))
    dst = edge_dst.astype(np.int64)
    core = dst // SHARD
    blk = (dst % SHARD) // 128

    # per-(core, block) stats to pick NLO/NHI
    nlo_need, nhi_need = 0, 0
    per_block = {}
    for c in range(NCORES):
        m = core == c
        for b in range(NBLK):
            mb = m & (blk == b)
            s_all, d_all = src[mb], dst[mb]
            order = np.argsort(s_all, kind="stable")
            per_block[(c, b)] = (s_all[order], d_all[order])

    # lo gather reads table chunk 0 (positions < POS0), hi reads chunk 1.
    def feasible(nlo, nhi):
        for (c, b), (s_all, _) in per_block.items():
            if int((s_all < POS0).sum()) > nlo * 128:
                return False
            if int((s_all >= POS0).sum()) > nhi * 128:
                return False
        return True

    nsub = None
    for tot in range(15, 22):
        ok = [
            (nlo, tot - nlo)
            for nlo in range(tot // 2, tot)
            if feasible(nlo, tot - nlo)
        ]
        if ok:
            nsub = tot
            nlo_need, nhi_need = ok[0]
            break
    assert nsub is not None, "no feasible (NLO, NHI) split"
    NLO, NHI = nlo_need, nhi_need
    NSUB = NLO + NHI
    ML = NLO * 8 + NHI * 8 + NSUB  # meta cols (int16)

    out = []
    for c in range(NCORES):
        meta = np.zeros((NBLK, 128, ML), np.int16)
        counts = np.zeros((NBLK, 2), np.int32)
        for b in range(NBLK):
            s_all, d_all = per_block[(c, b)]
            ncut = int((s_all < POS0).sum())
            lo_s, lo_d = s_all[:ncut], d_all[:ncut]
            hi_s, hi_d = s_all[ncut:] - POS0, d_all[ncut:]
            dstl = np.full((NSUB * 128,), PAD_DSTL, np.float32)
            for half, (ss, dd, cap, off) in enumerate(
                ((lo_s, lo_d, NLO, 0), (hi_s, hi_d, NHI, NLO))
            ):
                n = len(ss)
                assert n <= cap * 128, (c, b, half, n)
                sp = np.full(cap * 128, -1, np.int16)
                sp[:n] = ss.astype(np.int16)
                dl = dstl[off * 128 : (off + cap) * 128]
                dl[:n] = (dd - c * SHARD - b * 128).astype(np.float32)
                if n == 0:  # keep the gather ucode's count >= 1
                    sp[0], n = 0, 1
                counts[b, half] = n
                w = _wrap_idx(sp)
                if half == 0:
                    meta[b, :, 0 : NLO * 8] = w
                else:
                    meta[b, :, NLO * 8 : NLO * 8 + NHI * 8] = w
            # gathered row i lands at [p=i%128, j=i//128]
            dst_slot = dstl.reshape(NSUB, 128).T.astype(bfnp)  # [128, NSUB]
            meta[b, :, NLO * 8 + NHI * 8 :] = dst_slot.view(np.int16)
        out.append({"meta": meta, "counts": counts.reshape(-1)})
    return out, NLO, NHI


def _dense_phase(nc, tc, sb, ps, h_in, w_tiles, own_tab, tb_c0, tb_c1, used, h_cols):
    """own_tab = h_in @ W' (contiguous), also written into the padded
    chunk-major table halves tb_c0/tb_c1 that feed the AllGathers. h_in is a
    DRAM [SHARD, h_cols] bf16 tensor (row-major). Transpose-loads and table
    writes are batched 4 blocks at a time to keep the HWDGE queue short."""
    nk = h_cols // 128
    BB = 4  # blocks per batch; batches never straddle the chunk boundary
    t = 0
    while t < NBLK:
        lim = CH0_BLKS if t < CH0_BLKS else NBLK
        nb = min(BB, lim - t)
        hts = []
        for k in range(nk):
            ht = sb.tile([128, nb * 128], bf16, tag="ht", bufs=2, name=f"ht{k}")
            nc.sync.dma_start(
                out=ht[:],
                in_=h_in[t * 128 : (t + nb) * 128, k * 128 : (k + 1) * 128],
                transpose=True,
            )
            hts.append(ht)
        rows = sb.tile([128, nb, used], bf16, tag="drow", bufs=2)
        for j in range(nb):
            psd = ps.tile([128, used], f32, tag="psd", bufs=2)
            for k in range(nk):
                nc.tensor.matmul(
                    out=psd[:],
                    lhsT=hts[k][:, j * 128 : (j + 1) * 128],
                    rhs=w_tiles[k][:],
                    start=(k == 0),
                    stop=(k == nk - 1),
                )
            nc.scalar.activation(out=rows[:, j, :], in_=psd[:], func=AF.Copy)
        nc.sync.dma_start(
            out=own_tab[t * 128 : (t + nb) * 128, :].rearrange(
                "(j p) c -> p j c", j=nb
            ),
            in_=rows[:, 0:nb, :],
        )
        if t < CH0_BLKS:
            tb_c, r0 = tb_c0, t * 128
        else:
            tb_c, r0 = tb_c1, t * 128 - CH0
        nc.sync.dma_start(
            out=tb_c[r0 : r0 + nb * 128, 0:used].rearrange(
                "(j p) c -> p j c", j=nb
            ),
            in_=rows[:, 0:nb, :],
        )
        t += nb


def _edge_phase(
    nc,
    tc,
    sb,
    ps,
    gbufs,
    table_c0,
    table_c1,
    own_tab,
    t_meta,
    t_counts,
    counts_t,
    iota_t,
    ident_t,
    H,
    F,
    drow,
    used,
    bn_c_t,
    h_out,
    final,
    NLO,
    NHI,
):
    NSUB = NLO + NHI
    HF = H * F
    rcols = HF + H  # matmul rhs cols: [gw | ex]
    ML = NLO * 8 + NHI * 8 + NSUB

    r_lo = nc.gpsimd.alloc_register(f"cnt_lo_{nc.next_id()}")
    r_hi = nc.gpsimd.alloc_register(f"cnt_hi_{nc.next_id()}")
    for b in range(NBLK):
        qlo, qhi = (2 * b) % 4, (2 * b + 1) % 4
        meta = sb.tile([128, ML], i16, tag="meta", bufs=4)
        nc.sync.dma_start(out=meta[:], in_=t_meta[b])
        own = sb.tile([128, used], bf16, tag="own", bufs=3)
        nc.sync.dma_start(out=own[:], in_=own_tab[b * 128 : (b + 1) * 128, :])
        dstl = meta[:, NLO * 8 + NHI * 8 :].bitcast(bf16)

        nc.gpsimd.reg_load(r_lo, counts_t[0:1, 2 * b : 2 * b + 1])
        nc.gpsimd.reg_load(r_hi, counts_t[0:1, 2 * b + 1 : 2 * b + 2])
        cnt_lo, cnt_hi = r_lo, r_hi

        # pinned, pre-zeroed buffers: slots skipped by the runtime descriptor
        # count only ever expose older gathered rows (finite), never raw SBUF.
        G = gbufs[b % GBUFS]
        nc.gpsimd.dma_gather(
            out_ap=G[:, 0:NLO, :],
            in_ap=table_c0[:],
            idxs_ap=meta[:16, 0 : NLO * 8],
            num_idxs=NLO * 128,
            num_idxs_reg=cnt_lo,
            elem_size=drow,
            single_packet=SINGLE_PACKET,
            queue_num=qlo,
        )
        nc.gpsimd.dma_gather(
            out_ap=G[:, NLO:NSUB, :],
            in_ap=table_c1[:],
            idxs_ap=meta[:16, NLO * 8 : NLO * 8 + NHI * 8],
            num_idxs=NHI * 128,
            num_idxs_reg=cnt_hi,
            elem_size=drow,
            single_packet=SINGLE_PACKET,
            queue_num=qhi,
        )

        # one-hot selection matrices, U[e, j, slot] = (dst_local[e, j] == slot)
        U = sb.tile([128, NSUB, 128], bf16, tag="U", bufs=2)
        nc.vector.tensor_tensor(
            out=U[:],
            in0=iota_t[:, None, :].to_broadcast([128, NSUB, 128]),
            in1=dstl[:, :, None].to_broadcast([128, NSUB, 128]),
            op=OP.is_equal,
        )
        # transposed one-hots (for d expansion), via PE transpose in groups of 4
        Ut = sb.tile([128, NSUB, 128], bf16, tag="Ut", bufs=2)
        for g in range(0, NSUB, 4):
            n = min(4, NSUB - g)
            pst = ps.tile([128, 512], bf16, tag="pst", bufs=2)
            for k in range(n):
                nc.tensor.transpose(
                    out=pst[:, k * 128 : (k + 1) * 128],
                    in_=U[:, g + k, :],
                    identity=ident_t[:],
                )
            nc.scalar.activation(
                out=Ut[:, g : g + n, :],
                in_=pst[:, : n * 128].rearrange("p (j e) -> p j e", j=n),
                func=AF.Copy,
            )
        # d per edge: d_pe[e, h] = d_blk[dst_local[e], h]
        dblk = own[:, HF + H : HF + 2 * H]
        psd = ps.tile([128, NSUB * H], f32, tag="psdpe", bufs=2)
        for j in range(NSUB):
            nc.tensor.matmul(
                out=psd[:, j * H : (j + 1) * H],
                lhsT=Ut[:, j, :],
                rhs=dblk,
                start=True,
                stop=True,
            )
        # ex = exp(leaky_relu(s_src + d_dst)) == max(exp(x), exp(0.2 x))
        alpha = sb.tile([128, NSUB * H], f32, tag="alpha", bufs=2)
        nc.vector.tensor_tensor(
            out=alpha[:].rearrange("p (j h) -> p j h", j=NSUB),
            in0=G[:, :, HF : HF + H],
            in1=psd[:].rearrange("p (j h) -> p j h", j=NSUB),
            op=OP.add,
        )
        ex1 = sb.tile([128, NSUB * H], bf16, tag="ex1", bufs=2)
        nc.scalar.activation(out=ex1[:], in_=alpha[:], func=AF.Exp)
        ex2 = sb.tile([128, NSUB * H], bf16, tag="ex2", bufs=2)
        nc.scalar.activation(out=ex2[:], in_=alpha[:], func=AF.Exp, scale=NEG)
        # rhs tile for the psa accumulation: [xw*ex | ex]; the max lands
        # directly in the ex columns and the multiply re-reads them.
        gw = sb.tile([128, NSUB, rcols], bf16, tag="gw", bufs=2)
        nc.vector.tensor_tensor(
            out=gw[:, :, HF:rcols],
            in0=ex1[:].rearrange("p (j h) -> p j h", j=NSUB),
            in1=ex2[:].rearrange("p (j h) -> p j h", j=NSUB),
            op=OP.max,
        )
        nc.vector.tensor_tensor(
            out=gw[:, :, 0:HF].rearrange("p j (h f) -> p j h f", h=H),
            in0=G[:, :, 0:HF].rearrange("p j (h f) -> p j h f", h=H),
            in1=gw[:, :, HF:rcols][:, :, :, None].to_broadcast(
                [128, NSUB, H, F]
            ),
            op=OP.mult,
        )
        # self-loop contribution from the local contiguous table
        alph_s = sb.tile([128, H], f32, tag="alphs", bufs=2)
        nc.vector.tensor_tensor(
            out=alph_s[:],
            in0=own[:, HF : HF + H],
            in1=own[:, HF + H : HF + 2 * H],
            op=OP.add,
        )
        ex1s = sb.tile([128, H], bf16, tag="ex1s", bufs=2)
        nc.scalar.activation(out=ex1s[:], in_=alph_s[:], func=AF.Exp)
        ex2s = sb.tile([128, H], bf16, tag="ex2s", bufs=2)
        nc.scalar.activation(out=ex2s[:], in_=alph_s[:], func=AF.Exp, scale=NEG)
        rs = sb.tile([128, rcols], bf16, tag="rs", bufs=2)
        nc.vector.tensor_tensor(
            out=rs[:, HF:rcols], in0=ex1s[:], in1=ex2s[:], op=OP.max
        )
        nc.vector.tensor_tensor(
            out=rs[:, 0:HF].rearrange("p (h f) -> p h f", h=H),
            in0=own[:, 0:HF].rearrange("p (h f) -> p h f", h=H),
            in1=rs[:, HF:rcols][:, :, None].to_broadcast([128, H, F]),
            op=OP.mult,
        )
        # accumulate [num | den]: self-loop first, then the gathered sub-tiles
        psa = ps.tile([128, rcols], f32, tag="psa", bufs=2)
        nc.tensor.matmul(
            out=psa[:], lhsT=ident_t[:], rhs=rs[:], start=True, stop=False
        )
        for j in range(NSUB):
            nc.tensor.matmul(
                out=psa[:],
                lhsT=U[:, j, :],
                rhs=gw[:, j, :],
                start=False,
                stop=(j == NSUB - 1),
            )
        # normalize + affine + activation
        rden = sb.tile([128, H], f32, tag="rden", bufs=2)
        nc.vector.reciprocal_approx_fast(out=rden[:], in_=psa[:, HF:rcols])
        o1 = sb.tile([128, HF], f32, tag="o1", bufs=2)
        nc.vector.tensor_tensor(
            out=o1[:].rearrange("p (h f) -> p h f", h=H),
            in0=psa[:, 0:HF].rearrange("p (h f) -> p h f", h=H),
            in1=rden[:].to_broadcast([128, H, F]),
            op=OP.mult,
        )
        o3 = sb.tile([128, HF], f32, tag="o3", bufs=2)
        nc.vector.tensor_tensor(out=o3[:], in0=o1[:], in1=bn_c_t[:], op=OP.add)
        if final:
            outt = sb.tile([128, HF], f32, tag="outt", bufs=2)
            nc.scalar.activation(out=outt[:], in_=o3[:], func=AF.Sigmoid)
            nc.sync.dma_start(out=h_out[b * 128 : (b + 1) * 128, :], in_=outt[:])
        else:
            # elu(x) = max(x, exp(min(x, 0)) - 1); min(x,0) == -relu(-x)
            e1 = sb.tile([128, HF], f32, tag="e1", bufs=2)
            nc.scalar.activation(out=e1[:], in_=o3[:], func=AF.Relu, scale=-1.0)
            e2 = sb.tile([128, HF], f32, tag="e2", bufs=2)
            nc.scalar.activation(out=e2[:], in_=e1[:], func=AF.Exp, scale=-1.0)
            hb = sb.tile([128, HF], bf16, tag="hb", bufs=2)
            nc.vector.scalar_tensor_tensor(
                out=hb[:],
                in0=e2[:],
                scalar=-1.0,
                in1=o3[:],
                op0=OP.add,
                op1=OP.max,
            )
            nc.sync.dma_start(out=h_out[b * 128 : (b + 1) * 128, :], in_=hb[:])


def _build_program(NLO, NHI):
    nc = bacc.Bacc(
        "TRN2",
        target_bir_lowering=False,
        debug=False,
        num_devices=NCORES,
        num_swdge_queues=4,
    )
    HD = HEADS * HID
    NSUB = NLO + NHI
    ML = NLO * 8 + NHI * 8 + NSUB

    # --- inputs ---
    t_x = nc.dram_tensor("x_shard", [SHARD, DIN], bf16, kind="ExternalInput")
    t_w1 = nc.dram_tensor("W1p", [DIN, USED12], bf16, kind="ExternalInput")
    t_w2 = nc.dram_tensor("W2p", [HD, USED12], bf16, kind="ExternalInput")
    t_w3 = nc.dram_tensor("W3p", [HD, USED3], bf16, kind="ExternalInput")
    t_c1 = nc.dram_tensor("bn_c1", [128, HD], f32, kind="ExternalInput")
    t_c2 = nc.dram_tensor("bn_c2", [128, HD], f32, kind="ExternalInput")
    t_c3 = nc.dram_tensor("bn_c3", [128, DOUT], f32, kind="ExternalInput")
    t_iota = nc.dram_tensor("iota_bf", [128, 128], bf16, kind="ExternalInput")
    t_meta = nc.dram_tensor("meta", [NBLK, 128, ML], i16, kind="ExternalInput")
    t_counts = nc.dram_tensor("counts", [1, NBLK * 2], i32, kind="ExternalInput")
    t_out = nc.dram_tensor("out_shard", [SHARD, DOUT], f32, kind="ExternalOutput")

    with tile.TileContext(nc) as tc:
        with (
            tc.tile_pool(name="sb", bufs=2) as sb,
            tc.tile_pool(name="ps", bufs=2, space="PSUM") as ps,
            tc.tile_pool(name="dram", bufs=1, space="DRAM") as dr,
        ):
            # DRAM intermediates (pool tiles so Tile tracks dependencies)
            tbf = {
                (lyr, k): dr.tile(
                    [NCORES * (CH0 if k == 0 else CH1), DROW12 if lyr < 3 else DROW3],
                    bf16,
                    addr_space="Shared",
                    name=f"tb{lyr}_full{k}",
                )
                for lyr in (1, 2, 3)
                for k in (0, 1)
            }
            tbc = {
                (lyr, k): dr.tile(
                    [CH0 if k == 0 else CH1, DROW12 if lyr < 3 else DROW3],
                    bf16,
                    name=f"tb{lyr}_c{k}",
                )
                for lyr in (1, 2, 3)
                for k in (0, 1)
            }
            own1 = dr.tile([SHARD, USED12], bf16, name="own1")
            own2 = dr.tile([SHARD, USED12], bf16, name="own2")
            own3 = dr.tile([SHARD, USED3], bf16, name="own3")
            h2_own = dr.tile([SHARD, HD], bf16, name="h2_own")
            h3_own = dr.tile([SHARD, HD], bf16, name="h3_own")

            # constants
            iota_t = sb.tile([128, 128], bf16, tag="iota", bufs=1)
            nc.sync.dma_start(out=iota_t[:], in_=t_iota[:])
            ident_t = sb.tile([128, 128], bf16, tag="ident", bufs=1)
            make_identity(nc, ident_t[:])
            counts_t = sb.tile([1, NBLK * 2], i32, tag="counts", bufs=1)
            nc.sync.dma_start(out=counts_t[:], in_=t_counts[:])
            w1t = [sb.tile([128, USED12], bf16, tag="w1", bufs=1, name="w1t0")]
            nc.sync.dma_start(out=w1t[0][:], in_=t_w1[:])
            w2t = [sb.tile([128, USED12], bf16, tag=f"w2_{k}", bufs=1, name=f"w2t{k}") for k in range(2)]
            for k in range(2):
                nc.sync.dma_start(out=w2t[k][:], in_=t_w2[k * 128 : (k + 1) * 128, :])
            w3t = [sb.tile([128, USED3], bf16, tag=f"w3_{k}", bufs=1, name=f"w3t{k}") for k in range(2)]
            for k in range(2):
                nc.sync.dma_start(out=w3t[k][:], in_=t_w3[k * 128 : (k + 1) * 128, :])
            bn = {}
            for nm, t, w in (("c1", t_c1, HD), ("c2", t_c2, HD), ("c3", t_c3, DOUT)):
                bt = sb.tile([128, w], f32, tag=f"bn{nm}", bufs=1, name=f"bn{nm}")
                nc.sync.dma_start(out=bt[:], in_=t[:])
                bn[nm] = bt
            gb12 = [
                sb.tile([128, NSUB, DROW12], bf16, tag=f"G12_{i}", bufs=1, name=f"G12_{i}")
                for i in range(GBUFS)
            ]
            gb3 = [
                sb.tile([128, NSUB, DROW3], bf16, tag=f"G3_{i}", bufs=1, name=f"G3_{i}")
                for i in range(GBUFS)
            ]
            for gt in gb12 + gb3:
                nc.vector.memset(gt[:], 0.0)

            rg = [list(range(NCORES))]

            def ag_chunks(lyr):
                for k in (0, 1):
                    nc.gpsimd.collective_compute(
                        "AllGather", OP.bypass, replica_groups=rg,
                        ins=[tbc[(lyr, k)][:]], outs=[tbf[(lyr, k)][:]],
                    )

            # ---- layer 1 ----
            _dense_phase(
                nc, tc, sb, ps, t_x, w1t, own1, tbc[(1, 0)], tbc[(1, 1)],
                USED12, DIN,
            )
            ag_chunks(1)
            _edge_phase(
                nc, tc, sb, ps, gb12, tbf[(1, 0)], tbf[(1, 1)], own1, t_meta, t_counts, counts_t,
                iota_t, ident_t, HEADS, HID, DROW12, USED12, bn["c1"],
                h2_own, False, NLO, NHI,
            )
            # ---- layer 2 ----
            _dense_phase(
                nc, tc, sb, ps, h2_own, w2t, own2, tbc[(2, 0)], tbc[(2, 1)],
                USED12, HD,
            )
            ag_chunks(2)
            _edge_phase(
                nc, tc, sb, ps, gb12, tbf[(2, 0)], tbf[(2, 1)], own2, t_meta, t_counts, counts_t,
                iota_t, ident_t, HEADS, HID, DROW12, USED12, bn["c2"],
                h3_own, False, NLO, NHI,
            )
            # ---- layer 3 ----
            _dense_phase(
                nc, tc, sb, ps, h3_own, w3t, own3, tbc[(3, 0)], tbc[(3, 1)],
                USED3, HD,
            )
            ag_chunks(3)
            _edge_phase(
                nc, tc, sb, ps, gb3, tbf[(3, 0)], tbf[(3, 1)], own3, t_meta, t_counts, counts_t,
                iota_t, ident_t, 1, DOUT, DROW3, USED3, bn["c3"],
                t_out, True, NLO, NHI,
            )

    nc.compile()
    return nc


_CACHED = {}


def kernel(**inputs):
    x = np.asarray(inputs["x"], np.float32)
    edge_src = np.asarray(inputs["edge_src"], np.int32)
    edge_dst = np.asarray(inputs["edge_dst"], np.int32)

    xp = np.zeros((NPAD, DIN), np.float32)
    xp[:N] = x
    xb = xp.astype(bfnp)

    def aff(g, v, b, m, be):
        a = np.asarray(g, np.float32) / np.sqrt(np.asarray(v, np.float32) + EPS)
        c = (np.asarray(b, np.float32) - np.asarray(m, np.float32)) * a + np.asarray(
            be, np.float32
        )
        return a, c

    a1, c1 = aff(inputs["g1"], inputs["v1"], inputs["b1"], inputs["m1"], inputs["be1"])
    a2, c2 = aff(inputs["g2"], inputs["v2"], inputs["b2"], inputs["m2"], inputs["be2"])
    a3 = np.ones(DOUT, np.float32)
    c3 = np.asarray(inputs["b3"], np.float32)

    W1p = _build_Wp(
        np.asarray(inputs["W1"], np.float32),
        np.asarray(inputs["as1"], np.float32),
        np.asarray(inputs["ad1"], np.float32),
        a1,
    )
    W2p = _build_Wp(
        np.asarray(inputs["W2"], np.float32),
        np.asarray(inputs["as2"], np.float32),
        np.asarray(inputs["ad2"], np.float32),
        a2,
    )
    W3p = _build_Wp(
        np.asarray(inputs["W3"], np.float32),
        np.asarray(inputs["as3"], np.float32),
        np.asarray(inputs["ad3"], np.float32),
        a3,
    )

    edata, NLO, NHI = _prep_edges(edge_src, edge_dst)
    iota = np.tile(np.arange(128, dtype=np.float32), (128, 1)).astype(bfnp)

    key = (NLO, NHI)
    if _CACHED.get("key") != key:
        _CACHED["nc"] = _build_program(NLO, NHI)
        _CACHED["key"] = key
    nc = _CACHED["nc"]

    def bcast(v):
        return np.tile(np.asarray(v, np.float32), (128, 1))

    in_maps = []
    for c in range(NCORES):
        in_maps.append(
            {
                "x_shard": xb[c * SHARD : (c + 1) * SHARD],
                "W1p": W1p,
                "W2p": W2p,
                "W3p": W3p,
                "bn_c1": bcast(c1),
                "bn_c2": bcast(c2),
                "bn_c3": bcast(c3),
                "iota_bf": iota,
                "meta": edata[c]["meta"],
                "counts": edata[c]["counts"].reshape(1, -1),
            }
        )

    trace = bool(os.environ.get("GAT_TRACE"))
    res = run_bass_kernel_spmd(
        nc, in_maps, core_ids=list(range(NCORES)), trace=trace
    )
    if trace and res.exec_time_ns:
        print(f"HW exec time: {res.exec_time_ns} ns")
    out = np.concatenate([res.results[c]["out_shard"] for c in range(NCORES)], axis=0)
    return np.ascontiguousarray(out[:N]).astype(np.float32)


# revision 39
# speedup vs baseline: 2.2059x; 1.1028x over previous
"""GAT (3-layer, PyG GATConv semantics) on 8 Trainium2 NeuronCores.

Strategy (dst-node sharding):
- Nodes padded to 50176 = 8 * 6272; core c owns dst rows [c*6272, (c+1)*6272).
- Per layer:
  dense (sharded): table_shard = h_shard @ [W*bn_a | W@a_src | W@a_dst] in bf16
      (bn scale folded into W's feature columns on the host), written both at
      a 256B-multiple pitch (for dma_gather) and contiguous (for local reads).
  AllGather the padded table so every core can fetch arbitrary src rows.
  edge phase: edges bucketed by dst into 128-slot blocks. Self-loops are NOT
      gathered: their contribution comes from the local contiguous table via
      an identity-lhsT matmul. Remaining edges are packed lo/hi (int16 index
      range split), padded with trailing -1 indices which the gather ucode
      SKIPS -- the true per-block descriptor count is passed at runtime via
      value_load, so padding costs nothing on the DMA engines.
      Per block: batched dma_gather of src rows, one-hot selection matrices
      via is_equal(iota, dst_local), segment softmax without max-subtraction
      (logits are O(0.3)), ex-weighted features + ex accumulated into PSUM
      with one matmul per 128-edge sub-tile, then normalize / bn / activation
      fused on DVE+ACT (elu via one scalar_tensor_tensor).
- d[dst] per edge is produced on-chip: transpose the one-hot with the PE and
  multiply against the block's d rows.
"""
import os
import sys
import types

sys.path.insert(0, "/opt/trn_rl_repo")

import numpy as np
import ml_dtypes


def _install_ntff_shim():
    """Provide antenv.axon_hooks so run_bass_kernel_spmd(trace=True) works."""
    try:
        import antenv

        if "antenv.axon_hooks" in sys.modules:
            return
        mod = types.ModuleType("antenv.axon_hooks")
        mod._hook = None
        mod.set_axon_ntff_profile_hook = lambda h: setattr(mod, "_hook", h)
        mod.get_axon_ntff_profile_hook = lambda: mod._hook
        sys.modules["antenv.axon_hooks"] = mod
        antenv.axon_hooks = mod
        from trn_agent_boot.trn_boot import _ntff_profile_via_ctypes

        hook = _ntff_profile_via_ctypes("/opt/axon/libaxon_pjrt.so")
        if hook is not None:
            mod.set_axon_ntff_profile_hook(hook)
    except Exception:
        pass


_install_ntff_shim()

import concourse.bass as bass
import concourse.bacc as bacc
import concourse.mybir as mybir
import concourse.tile as tile
from concourse.bass_utils import run_bass_kernel_spmd
from concourse.masks import make_identity

bfnp = ml_dtypes.bfloat16
f32 = mybir.dt.float32
bf16 = mybir.dt.bfloat16
i16 = mybir.dt.int16
i32 = mybir.dt.int32
AF = mybir.ActivationFunctionType
OP = mybir.AluOpType

N, E = 50000, 800000
DIN, HID, HEADS, DOUT = 128, 32, 8, 16
NEG = 0.2
EPS = 1e-5

NCORES = 8
SHARD = 6272
NPAD = NCORES * SHARD  # 50176
NBLK = SHARD // 128  # 49
LOCUT = 32768
HI_BASE = NPAD - LOCUT  # 17408

# table row pitches (bf16 cols; byte pitch must be a multiple of 256)
DROW12, USED12 = 384, 272  # [xw(256) | s(8) | d(8) | pad]
DROW3, USED3 = 128, 18  # [xw(16) | s(1) | d(1) | pad]
PAD_DSTL = 200.0  # one-hot miss marker for padding edges
GBUFS = 5
SINGLE_PACKET = False

# The full table is laid out chunk-major: chunk 0 holds every core's shard
# rows [0, CH0), chunk 1 the rest. Each chunk is then a CONTIGUOUS AllGather
# destination, and chunk 0's collective overlaps chunk 1's dense phase.
CH0_BLKS = 25
CH0 = CH0_BLKS * 128  # 3200
CH1 = SHARD - CH0  # 3072
POS0 = NCORES * CH0  # 25600


def _pos(g):
    """global node id -> chunk-major position in the gathered table."""
    c, r = g // SHARD, g % SHARD
    return np.where(r < CH0, c * CH0 + r, POS0 + c * CH1 + (r - CH0))


def _build_Wp(W, a_s, a_d, bn_a):
    H, F = a_s.shape
    Ws = np.stack([W[:, h * F : (h + 1) * F] @ a_s[h] for h in range(H)], axis=1)
    Wd = np.stack([W[:, h * F : (h + 1) * F] @ a_d[h] for h in range(H)], axis=1)
    return np.concatenate([W * bn_a[None, :], Ws, Wd], axis=1).astype(bfnp)


def _wrap_idx(flat):
    """int16 list -> [128, len/16] wrapped in 16 partitions, replicated x8."""
    n = len(flat)
    assert n % 16 == 0
    w = flat.reshape(n // 16, 16).T  # [16, n/16]
    return np.tile(w, (8, 1)).astype(np.int16)


def _prep_edges(edge_src, edge_dst):
    """Bucket non-self-loop edges by (core, block); returns per-core packed
    meta arrays (idx_lo | idx_hi | dstl) plus per-block valid counts, and the
    (NLO, NHI) sub-tile split chosen from the data."""
    src = _pos(edge_src.astype(np.int64))
    dst = edge_dst.astype(np.int64)
    core = dst // SHARD
    blk = (dst % SHARD) // 128

    # per-(core, block) stats to pick NLO/NHI
    nlo_need, nhi_need = 0, 0
    per_block = {}
    for c in range(NCORES):
        m = core == c
        for b in range(NBLK):
            mb = m & (blk == b)
            s_all, d_all = src[mb], dst[mb]
            order = np.argsort(s_all, kind="stable")
            per_block[(c, b)] = (s_all[order], d_all[order])

    # lo gather reads table chunk 0 (positions < POS0), hi reads chunk 1.
    def feasible(nlo, nhi):
        for (c, b), (s_all, _) in per_block.items():
            if int((s_all < POS0).sum()) > nlo * 128:
                return False
            if int((s_all >= POS0).sum()) > nhi * 128:
                return False
        return True

    nsub = None
    for tot in range(15, 22):
        ok = [
            (nlo, tot - nlo)
            for nlo in range(tot // 2, tot)
            if feasible(nlo, tot - nlo)
        ]
        if ok:
            nsub = tot
            nlo_need, nhi_need = ok[0]
            break
    assert nsub is not None, "no feasible (NLO, NHI) split"
    NLO, NHI = nlo_need, nhi_need
    NSUB = NLO + NHI
    ML = NLO * 8 + NHI * 8 + NSUB  # meta cols (int16)

    out = []
    for c in range(NCORES):
        meta = np.zeros((NBLK, 128, ML), np.int16)
        counts = np.zeros((NBLK, 2), np.int32)
        for b in range(NBLK):
            s_all, d_all = per_block[(c, b)]
            ncut = int((s_all < POS0).sum())
            lo_s, lo_d = s_all[:ncut], d_all[:ncut]
            hi_s, hi_d = s_all[ncut:] - POS0, d_all[ncut:]
            dstl = np.full((NSUB * 128,), PAD_DSTL, np.float32)
            for half, (ss, dd, cap, off) in enumerate(
                ((lo_s, lo_d, NLO, 0), (hi_s, hi_d, NHI, NLO))
            ):
                n = len(ss)
                assert n <= cap * 128, (c, b, half, n)
                sp = np.full(cap * 128, -1, np.int16)
                sp[:n] = ss.astype(np.int16)
                dl = dstl[off * 128 : (off + cap) * 128]
                dl[:n] = (dd - c * SHARD - b * 128).astype(np.float32)
                if n == 0:  # keep the gather ucode's count >= 1
                    sp[0], n = 0, 1
                counts[b, half] = n
                w = _wrap_idx(sp)
                if half == 0:
                    meta[b, :, 0 : NLO * 8] = w
                else:
                    meta[b, :, NLO * 8 : NLO * 8 + NHI * 8] = w
            # gathered row i lands at [p=i%128, j=i//128]
            dst_slot = dstl.reshape(NSUB, 128).T.astype(bfnp)  # [128, NSUB]
            meta[b, :, NLO * 8 + NHI * 8 :] = dst_slot.view(np.int16)
        out.append({"meta": meta, "counts": counts.reshape(-1)})
    return out, NLO, NHI


def _dense_phase(nc, tc, sb, ps, h_in, w_tiles, own_tab, tb_c0, tb_c1, used, h_cols):
    """own_tab = h_in @ W' (contiguous), also written into the padded
    chunk-major table halves tb_c0/tb_c1 that feed the AllGathers. h_in is a
    DRAM [SHARD, h_cols] bf16 tensor (row-major). Transpose-loads and table
    writes are batched 4 blocks at a time to keep the HWDGE queue short."""
    nk = h_cols // 128
    BB = 4  # blocks per batch; batches never straddle the chunk boundary
    t = 0
    while t < NBLK:
        lim = CH0_BLKS if t < CH0_BLKS else NBLK
        nb = min(BB, lim - t)
        hts = []
        for k in range(nk):
            ht = sb.tile([128, nb * 128], bf16, tag="ht", bufs=2, name=f"ht{k}")
            nc.sync.dma_start(
                out=ht[:],
                in_=h_in[t * 128 : (t + nb) * 128, k * 128 : (k + 1) * 128],
                transpose=True,
            )
            hts.append(ht)
        rows = sb.tile([128, nb, used], bf16, tag="drow", bufs=2)
        for j in range(nb):
            psd = ps.tile([128, used], f32, tag="psd", bufs=2)
            for k in range(nk):
                nc.tensor.matmul(
                    out=psd[:],
                    lhsT=hts[k][:, j * 128 : (j + 1) * 128],
                    rhs=w_tiles[k][:],
                    start=(k == 0),
                    stop=(k == nk - 1),
                )
            nc.scalar.activation(out=rows[:, j, :], in_=psd[:], func=AF.Copy)
        nc.sync.dma_start(
            out=own_tab[t * 128 : (t + nb) * 128, :].rearrange(
                "(j p) c -> p j c", j=nb
            ),
            in_=rows[:, 0:nb, :],
        )
        if t < CH0_BLKS:
            tb_c, r0 = tb_c0, t * 128
        else:
            tb_c, r0 = tb_c1, t * 128 - CH0
        nc.sync.dma_start(
            out=tb_c[r0 : r0 + nb * 128, 0:used].rearrange(
                "(j p) c -> p j c", j=nb
            ),
            in_=rows[:, 0:nb, :],
        )
        t += nb


def _edge_phase(
    nc,
    tc,
    sb,
    ps,
    gbufs,
    table_c0,
    table_c1,
    own_tab,
    t_meta,
    t_counts,
    counts_t,
    iota_t,
    ident_t,
    H,
    F,
    drow,
    used,
    bn_c_t,
    h_out,
    final,
    NLO,
    NHI,
):
    NSUB = NLO + NHI
    HF = H * F
    rcols = HF + H  # matmul rhs cols: [gw | ex]
    ML = NLO * 8 + NHI * 8 + NSUB

    r_lo = nc.gpsimd.alloc_register(f"cnt_lo_{nc.next_id()}")
    r_hi = nc.gpsimd.alloc_register(f"cnt_hi_{nc.next_id()}")
    for b in range(NBLK):
        qlo, qhi = (2 * b) % 4, (2 * b + 1) % 4
        meta = sb.tile([128, ML], i16, tag="meta", bufs=5)
        nc.sync.dma_start(out=meta[:], in_=t_meta[b])
        own = sb.tile([128, used], bf16, tag="own", bufs=4)
        nc.sync.dma_start(out=own[:], in_=own_tab[b * 128 : (b + 1) * 128, :])
        dstl = meta[:, NLO * 8 + NHI * 8 :].bitcast(bf16)

        nc.gpsimd.reg_load(r_lo, counts_t[0:1, 2 * b : 2 * b + 1])
        nc.gpsimd.reg_load(r_hi, counts_t[0:1, 2 * b + 1 : 2 * b + 2])
        cnt_lo, cnt_hi = r_lo, r_hi

        # pinned, pre-zeroed buffers: slots skipped by the runtime descriptor
        # count only ever expose older gathered rows (finite), never raw SBUF.
        G = gbufs[b % GBUFS]
        nc.gpsimd.dma_gather(
            out_ap=G[:, 0:NLO, :],
            in_ap=table_c0[:],
            idxs_ap=meta[:16, 0 : NLO * 8],
            num_idxs=NLO * 128,
            num_idxs_reg=cnt_lo,
            elem_size=drow,
            single_packet=SINGLE_PACKET,
            queue_num=qlo,
        )
        nc.gpsimd.dma_gather(
            out_ap=G[:, NLO:NSUB, :],
            in_ap=table_c1[:],
            idxs_ap=meta[:16, NLO * 8 : NLO * 8 + NHI * 8],
            num_idxs=NHI * 128,
            num_idxs_reg=cnt_hi,
            elem_size=drow,
            single_packet=SINGLE_PACKET,
            queue_num=qhi,
        )

        # one-hot selection matrices, U[e, j, slot] = (dst_local[e, j] == slot)
        U = sb.tile([128, NSUB, 128], bf16, tag="U", bufs=3)
        nc.vector.tensor_tensor(
            out=U[:],
            in0=iota_t[:, None, :].to_broadcast([128, NSUB, 128]),
            in1=dstl[:, :, None].to_broadcast([128, NSUB, 128]),
            op=OP.is_equal,
        )
        # transposed one-hots (for d expansion), via PE transpose in groups of 4
        Ut = sb.tile([128, NSUB, 128], bf16, tag="Ut", bufs=3)
        for g in range(0, NSUB, 4):
            n = min(4, NSUB - g)
            pst = ps.tile([128, 512], bf16, tag="pst", bufs=2)
            for k in range(n):
                nc.tensor.transpose(
                    out=pst[:, k * 128 : (k + 1) * 128],
                    in_=U[:, g + k, :],
                    identity=ident_t[:],
                )
            nc.scalar.activation(
                out=Ut[:, g : g + n, :],
                in_=pst[:, : n * 128].rearrange("p (j e) -> p j e", j=n),
                func=AF.Copy,
            )
        # d per edge: d_pe[e, h] = d_blk[dst_local[e], h]
        dblk = own[:, HF + H : HF + 2 * H]
        psd = ps.tile([128, NSUB * H], f32, tag="psdpe", bufs=2)
        for j in range(NSUB):
            nc.tensor.matmul(
                out=psd[:, j * H : (j + 1) * H],
                lhsT=Ut[:, j, :],
                rhs=dblk,
                start=True,
                stop=True,
            )
        # ex = exp(leaky_relu(s_src + d_dst)) == max(exp(x), exp(0.2 x))
        alpha = sb.tile([128, NSUB * H], f32, tag="alpha", bufs=3)
        nc.vector.tensor_tensor(
            out=alpha[:].rearrange("p (j h) -> p j h", j=NSUB),
            in0=G[:, :, HF : HF + H],
            in1=psd[:].rearrange("p (j h) -> p j h", j=NSUB),
            op=OP.add,
        )
        ex1 = sb.tile([128, NSUB * H], bf16, tag="ex1", bufs=3)
        nc.scalar.activation(out=ex1[:], in_=alpha[:], func=AF.Exp)
        ex2 = sb.tile([128, NSUB * H], bf16, tag="ex2", bufs=3)
        nc.scalar.activation(out=ex2[:], in_=alpha[:], func=AF.Exp, scale=NEG)
        # rhs tile for the psa accumulation: [xw*ex | ex]; the max lands
        # directly in the ex columns and the multiply re-reads them.
        gw = sb.tile([128, NSUB, rcols], bf16, tag="gw", bufs=3)
        nc.vector.tensor_tensor(
            out=gw[:, :, HF:rcols],
            in0=ex1[:].rearrange("p (j h) -> p j h", j=NSUB),
            in1=ex2[:].rearrange("p (j h) -> p j h", j=NSUB),
            op=OP.max,
        )
        nc.vector.tensor_tensor(
            out=gw[:, :, 0:HF].rearrange("p j (h f) -> p j h f", h=H),
            in0=G[:, :, 0:HF].rearrange("p j (h f) -> p j h f", h=H),
            in1=gw[:, :, HF:rcols][:, :, :, None].to_broadcast(
                [128, NSUB, H, F]
            ),
            op=OP.mult,
        )
        # self-loop contribution from the local contiguous table
        alph_s = sb.tile([128, H], f32, tag="alphs", bufs=3)
        nc.vector.tensor_tensor(
            out=alph_s[:],
            in0=own[:, HF : HF + H],
            in1=own[:, HF + H : HF + 2 * H],
            op=OP.add,
        )
        ex1s = sb.tile([128, H], bf16, tag="ex1s", bufs=3)
        nc.scalar.activation(out=ex1s[:], in_=alph_s[:], func=AF.Exp)
        ex2s = sb.tile([128, H], bf16, tag="ex2s", bufs=3)
        nc.scalar.activation(out=ex2s[:], in_=alph_s[:], func=AF.Exp, scale=NEG)
        rs = sb.tile([128, rcols], bf16, tag="rs", bufs=3)
        nc.vector.tensor_tensor(
            out=rs[:, HF:rcols], in0=ex1s[:], in1=ex2s[:], op=OP.max
        )
        nc.vector.tensor_tensor(
            out=rs[:, 0:HF].rearrange("p (h f) -> p h f", h=H),
            in0=own[:, 0:HF].rearrange("p (h f) -> p h f", h=H),
            in1=rs[:, HF:rcols][:, :, None].to_broadcast([128, H, F]),
            op=OP.mult,
        )
        # accumulate [num | den]: self-loop first, then the gathered sub-tiles
        psa = ps.tile([128, rcols], f32, tag="psa", bufs=2)
        nc.tensor.matmul(
            out=psa[:], lhsT=ident_t[:], rhs=rs[:], start=True, stop=False
        )
        for j in range(NSUB):
            nc.tensor.matmul(
                out=psa[:],
                lhsT=U[:, j, :],
                rhs=gw[:, j, :],
                start=False,
                stop=(j == NSUB - 1),
            )
        # normalize + affine + activation
        rden = sb.tile([128, H], f32, tag="rden", bufs=3)
        nc.vector.reciprocal_approx_fast(out=rden[:], in_=psa[:, HF:rcols])
        o1 = sb.tile([128, HF], f32, tag="o1", bufs=3)
        nc.vector.tensor_tensor(
            out=o1[:].rearrange("p (h f) -> p h f", h=H),
            in0=psa[:, 0:HF].rearrange("p (h f) -> p h f", h=H),
            in1=rden[:].to_broadcast([128, H, F]),
            op=OP.mult,
        )
        o3 = sb.tile([128, HF], f32, tag="o3", bufs=3)
        nc.vector.tensor_tensor(out=o3[:], in0=o1[:], in1=bn_c_t[:], op=OP.add)
        if final:
            outt = sb.tile([128, HF], f32, tag="outt", bufs=3)
            nc.scalar.activation(out=outt[:], in_=o3[:], func=AF.Sigmoid)
            nc.sync.dma_start(out=h_out[b * 128 : (b + 1) * 128, :], in_=outt[:])
        else:
            # elu(x) = max(x, exp(min(x, 0)) - 1); min(x,0) == -relu(-x)
            e1 = sb.tile([128, HF], f32, tag="e1", bufs=3)
            nc.scalar.activation(out=e1[:], in_=o3[:], func=AF.Relu, scale=-1.0)
            e2 = sb.tile([128, HF], f32, tag="e2", bufs=3)
            nc.scalar.activation(out=e2[:], in_=e1[:], func=AF.Exp, scale=-1.0)
            hb = sb.tile([128, HF], bf16, tag="hb", bufs=3)
            nc.vector.scalar_tensor_tensor(
                out=hb[:],
                in0=e2[:],
                scalar=-1.0,
                in1=o3[:],
                op0=OP.add,
                op1=OP.max,
            )
            nc.sync.dma_start(out=h_out[b * 128 : (b + 1) * 128, :], in_=hb[:])


def _build_program(NLO, NHI):
    nc = bacc.Bacc(
        "TRN2",
        target_bir_lowering=False,
        debug=False,
        num_devices=NCORES,
        num_swdge_queues=4,
    )
    HD = HEADS * HID
    NSUB = NLO + NHI
    ML = NLO * 8 + NHI * 8 + NSUB

    # --- inputs ---
    t_x = nc.dram_tensor("x_shard", [SHARD, DIN], bf16, kind="ExternalInput")
    t_w1 = nc.dram_tensor("W1p", [DIN, USED12], bf16, kind="ExternalInput")
    t_w2 = nc.dram_tensor("W2p", [HD, USED12], bf16, kind="ExternalInput")
    t_w3 = nc.dram_tensor("W3p", [HD, USED3], bf16, kind="ExternalInput")
    t_c1 = nc.dram_tensor("bn_c1", [128, HD], f32, kind="ExternalInput")
    t_c2 = nc.dram_tensor("bn_c2", [128, HD], f32, kind="ExternalInput")
    t_c3 = nc.dram_tensor("bn_c3", [128, DOUT], f32, kind="ExternalInput")
    t_iota = nc.dram_tensor("iota_bf", [128, 128], bf16, kind="ExternalInput")
    t_meta = nc.dram_tensor("meta", [NBLK, 128, ML], i16, kind="ExternalInput")
    t_counts = nc.dram_tensor("counts", [1, NBLK * 2], i32, kind="ExternalInput")
    t_out = nc.dram_tensor("out_shard", [SHARD, DOUT], f32, kind="ExternalOutput")

    with tile.TileContext(nc) as tc:
        with (
            tc.tile_pool(name="sb", bufs=2) as sb,
            tc.tile_pool(name="ps", bufs=2, space="PSUM") as ps,
            tc.tile_pool(name="dram", bufs=1, space="DRAM") as dr,
        ):
            # DRAM intermediates (pool tiles so Tile tracks dependencies)
            tbf = {
                (lyr, k): dr.tile(
                    [NCORES * (CH0 if k == 0 else CH1), DROW12 if lyr < 3 else DROW3],
                    bf16,
                    addr_space="Shared",
                    name=f"tb{lyr}_full{k}",
                )
                for lyr in (1, 2, 3)
                for k in (0, 1)
            }
            tbc = {
                (lyr, k): dr.tile(
                    [CH0 if k == 0 else CH1, DROW12 if lyr < 3 else DROW3],
                    bf16,
                    name=f"tb{lyr}_c{k}",
                )
                for lyr in (1, 2, 3)
                for k in (0, 1)
            }
            own1 = dr.tile([SHARD, USED12], bf16, name="own1")
            own2 = dr.tile([SHARD, USED12], bf16, name="own2")
            own3 = dr.tile([SHARD, USED3], bf16, name="own3")
            h2_own = dr.tile([SHARD, HD], bf16, name="h2_own")
            h3_own = dr.tile([SHARD, HD], bf16, name="h3_own")

            # constants
            iota_t = sb.tile([128, 128], bf16, tag="iota", bufs=1)
            nc.sync.dma_start(out=iota_t[:], in_=t_iota[:])
            ident_t = sb.tile([128, 128], bf16, tag="ident", bufs=1)
            make_identity(nc, ident_t[:])
            counts_t = sb.tile([1, NBLK * 2], i32, tag="counts", bufs=1)
            nc.sync.dma_start(out=counts_t[:], in_=t_counts[:])
            w1t = [sb.tile([128, USED12], bf16, tag="w1", bufs=1, name="w1t0")]
            nc.sync.dma_start(out=w1t[0][:], in_=t_w1[:])
            w2t = [sb.tile([128, USED12], bf16, tag=f"w2_{k}", bufs=1, name=f"w2t{k}") for k in range(2)]
            for k in range(2):
                nc.sync.dma_start(out=w2t[k][:], in_=t_w2[k * 128 : (k + 1) * 128, :])
            w3t = [sb.tile([128, USED3], bf16, tag=f"w3_{k}", bufs=1, name=f"w3t{k}") for k in range(2)]
            for k in range(2):
                nc.sync.dma_start(out=w3t[k][:], in_=t_w3[k * 128 : (k + 1) * 128, :])
            bn = {}
            for nm, t, w in (("c1", t_c1, HD), ("c2", t_c2, HD), ("c3", t_c3, DOUT)):
                bt = sb.tile([128, w], f32, tag=f"bn{nm}", bufs=1, name=f"bn{nm}")
                nc.sync.dma_start(out=bt[:], in_=t[:])
                bn[nm] = bt
            gb12 = [
                sb.tile([128, NSUB, DROW12], bf16, tag=f"G12_{i}", bufs=1, name=f"G12_{i}")
                for i in range(GBUFS)
            ]
            gb3 = [
                sb.tile([128, NSUB, DROW3], bf16, tag=f"G3_{i}", bufs=1, name=f"G3_{i}")
                for i in range(GBUFS)
            ]
            for gt in gb12 + gb3:
                nc.vector.memset(gt[:], 0.0)

            rg = [list(range(NCORES))]

            def ag_chunks(lyr):
                for k in (0, 1):
                    nc.gpsimd.collective_compute(
                        "AllGather", OP.bypass, replica_groups=rg,
                        ins=[tbc[(lyr, k)][:]], outs=[tbf[(lyr, k)][:]],
                    )

            # ---- layer 1 ----
            _dense_phase(
                nc, tc, sb, ps, t_x, w1t, own1, tbc[(1, 0)], tbc[(1, 1)],
                USED12, DIN,
            )
            ag_chunks(1)
            _edge_phase(
                nc, tc, sb, ps, gb12, tbf[(1, 0)], tbf[(1, 1)], own1, t_meta, t_counts, counts_t,
                iota_t, ident_t, HEADS, HID, DROW12, USED12, bn["c1"],
                h2_own, False, NLO, NHI,
            )
            # ---- layer 2 ----
            _dense_phase(
                nc, tc, sb, ps, h2_own, w2t, own2, tbc[(2, 0)], tbc[(2, 1)],
                USED12, HD,
            )
            ag_chunks(2)
            _edge_phase(
                nc, tc, sb, ps, gb12, tbf[(2, 0)], tbf[(2, 1)], own2, t_meta, t_counts, counts_t,
                iota_t, ident_t, HEADS, HID, DROW12, USED12, bn["c2"],
                h3_own, False, NLO, NHI,
            )
            # ---- layer 3 ----
            _dense_phase(
                nc, tc, sb, ps, h3_own, w3t, own3, tbc[(3, 0)], tbc[(3, 1)],
                USED3, HD,
            )
            ag_chunks(3)
            _edge_phase(
                nc, tc, sb, ps, gb3, tbf[(3, 0)], tbf[(3, 1)], own3, t_meta, t_counts, counts_t,
                iota_t, ident_t, 1, DOUT, DROW3, USED3, bn["c3"],
                t_out, True, NLO, NHI,
            )

    nc.compile()
    return nc


_CACHED = {}


def kernel(**inputs):
    x = np.asarray(inputs["x"], np.float32)
    edge_src = np.asarray(inputs["edge_src"], np.int32)
    edge_dst = np.asarray(inputs["edge_dst"], np.int32)

    xp = np.zeros((NPAD, DIN), np.float32)
    xp[:N] = x
    xb = xp.astype(bfnp)

    def aff(g, v, b, m, be):
        a = np.asarray(g, np.float32) / np.sqrt(np.asarray(v, np.float32) + EPS)
        c = (np.asarray(b, np.float32) - np.asarray(m, np.float32)) * a + np.asarray(
            be, np.float32
        )
        return a, c

    a1, c1 = aff(inputs["g1"], inputs["v1"], inputs["b1"], inputs["m1"], inputs["be1"])
    a2, c2 = aff(inputs["g2"], inputs["v2"], inputs["b2"], inputs["m2"], inputs["be2"])
    a3 = np.ones(DOUT, np.float32)
    c3 = np.asarray(inputs["b3"], np.float32)

    W1p = _build_Wp(
        np.asarray(inputs["W1"], np.float32),
        np.asarray(inputs["as1"], np.float32),
        np.asarray(inputs["ad1"], np.float32),
        a1,
    )
    W2p = _build_Wp(
        np.asarray(inputs["W2"], np.float32),
        np.asarray(inputs["as2"], np.float32),
        np.asarray(inputs["ad2"], np.float32),
        a2,
    )
    W3p = _build_Wp(
        np.asarray(inputs["W3"], np.float32),
        np.asarray(inputs["as3"], np.float32),
        np.asarray(inputs["ad3"], np.float32),
        a3,
    )

    edata, NLO, NHI = _prep_edges(edge_src, edge_dst)
    iota = np.tile(np.arange(128, dtype=np.float32), (128, 1)).astype(bfnp)

    key = (NLO, NHI)
    if _CACHED.get("key") != key:
        _CACHED["nc"] = _build_program(NLO, NHI)
        _CACHED["key"] = key
    nc = _CACHED["nc"]

    def bcast(v):
        return np.tile(np.asarray(v, np.float32), (128, 1))

    in_maps = []
    for c in range(NCORES):
        in_maps.append(
            {
                "x_shard": xb[c * SHARD : (c + 1) * SHARD],
                "W1p": W1p,
                "W2p": W2p,
                "W3p": W3p,
                "bn_c1": bcast(c1),
                "bn_c2": bcast(c2),
                "bn_c3": bcast(c3),
                "iota_bf": iota,
                "meta": edata[c]["meta"],
                "counts": edata[c]["counts"].reshape(1, -1),
            }
        )

    trace = bool(os.environ.get("GAT_TRACE"))
    res = run_bass_kernel_spmd(
        nc, in_maps, core_ids=list(range(NCORES)), trace=trace
    )
    if trace and res.exec_time_ns:
        print(f"HW exec time: {res.exec_time_ns} ns")
    out = np.concatenate([res.results[c]["out_shard"] for c in range(NCORES)], axis=0)
    return np.ascontiguousarray(out[:N]).astype(np.float32)
